# revision 23
# baseline (speedup 1.0000x reference)
"""Ernie4 decoder layer (RMSNorm + GQA attention + shared expert + 16-expert
top-2 MoE) on 8 Trainium2 NeuronCores.

v3 (pipelined collectives):
  - Attention head-parallel, processed query-block-major: per 128-token
    block both heads' scores/softmax/AV and the o_proj run immediately,
    feeding 4 token-chunked ReduceScatters that fire DURING attention.
    Token ownership becomes permuted (32-row shards per chunk); the host
    permutes hid_slice in and unpermutes outputs.
  - AG splits in two column chunks: AGx1 (x lo-half) fires right after the
    norm; AGx2 (x hi-half | router payload) after the router. x^T build and
    the shared-expert gate pass consume the halves progressively.
  - Expert capacity 192; gate/up I-partitioned (no h transposes); expert
    weight-scale (wcol) gathers run before expert compute so the gpsimd
    queue never blocks the down-projections.
  - Down-proj weights load during expert gate/up; down-projections run
    column-half-outer feeding 2 chunked ReduceScatters so RS2a overlaps
    the second half's compute.
"""
import sys
sys.path.insert(0, "/opt/trn_rl_repo")

import numpy as np

import concourse.bass as bass
import concourse.bacc as bacc
import concourse.tile as tile
import concourse.mybir as mybir
from concourse import bass_utils

dt = mybir.dt
F32 = dt.float32
F16 = dt.float16
I32 = dt.int32
AF = mybir.ActivationFunctionType
ALU = mybir.AluOpType
AX = mybir.AxisListType

T, H, NH, NKV, D = 1024, 2048, 16, 4, 128
E, I, IS = 16, 1024, 2048
ISC = IS // 8
EPS = 1e-6
THETA = 10000.0
NCN = 8
P = 128
TB = T // P
HC = H // P
IP = I // P
CAP = 192               # per-expert compute capacity (rank mask)
SL = 256                # per-expert list-slot spacing (square layouts)
NCH = 2                 # RS1 token chunks
CH = T // NCH           # 256 tokens per chunk
SH = CH // NCN          # 32-row per-core shard per chunk
WP = 3 * E              # router payload width
BIG = 1.0e6
BIG2 = 30000.0
NEG = -30000.0
RG = [list(range(NCN))]


def _emit(nc, tc):
    ex = {}
    for name, shape, d in [
        ("hid", [T, H], F16), ("hid_slice", [P, H], F32),
        ("w_qkv_pk", [P, HC * 512], F16),
        ("wo0", [D, H], F16), ("wo1", [D, H], F16),
        ("cosq", [D, T], F16), ("sinq", [D, T], F16),
        ("cosk", [D, T], F16), ("sink", [D, T], F16),
        ("permh", [P, P], F16), ("identh_in", [P, P], F16),
        ("identr_in", [P, P], F32), ("diagmask", [P, P], F16),
        ("gate_w_pk", [P, HC * E], F32), ("gate_b", [P, E], F32),
        ("emask0", [P, E], F32), ("emask1", [P, E], F32),
        ("ut_in", [P, P], F16), ("slb_in", [8, TB * P], F16),
        ("bcast127", [P, P], F16),
        ("ws_g_pk", [P, HC * ISC], F16), ("ws_u_pk", [P, HC * ISC], F16),
        ("ws_d", [ISC, H], F16),
        ("we_g", [2, H, I], F16), ("we_u", [2, H, I], F16),
        ("we_d", [2, I, H], F16),
    ]:
        ex[name] = nc.dram_tensor(name, shape, d, kind="ExternalInput").ap()
    out_slice = nc.dram_tensor("out_slice", [P, H], F16, kind="ExternalOutput").ap()
    res_slice = nc.dram_tensor("res_slice", [P, H], F32, kind="ExternalOutput").ap()

    with tc.tile_pool(name="pp", bufs=1) as pp, \
         tc.tile_pool(name="dram", bufs=1, space="DRAM") as dram:
        rs1_in = [dram.tile([CH, H], F16, tag=f"rs1i{q}", name=f"rs1i{q}")
                  for q in range(NCH)]
        rs1_out = [dram.tile([SH, H], F16, tag=f"rs1o{q}", name=f"rs1o{q}")
                   for q in range(NCH)]
        agx1_in = dram.tile([P, H // 2 + WP], F16)
        agx2_in = dram.tile([P, H // 2], F16)
        x_tmA = dram.tile([T, H // 2 + WP], F16, addr_space="Shared")
        x_tmB = dram.tile([T, H // 2], F16, addr_space="Shared")
        tok_lists = dram.tile([2 * SL, 1], I32)
        rs2_in = [dram.tile([T, H // 2], F16, tag=f"rs2i{nn}",
                            name=f"rs2i{nn}") for nn in range(2)]
        rs2_out = [dram.tile([P, H // 2], F16, tag=f"rs2o{nn}",
                             name=f"rs2o{nn}") for nn in range(2)]

        identh = pp.tile([P, P], F16)
        nc.sync.dma_start(identh[:], ex["identh_in"][:])
        identf = pp.tile([P, P], F32)
        nc.sync.dma_start(identf[:], ex["identr_in"][:])
        eps_t = pp.tile([P, 1], F32)
        nc.vector.memset(eps_t[:], EPS)

        # ======== persistent weight pool (prefetched during attention) ====
        with tc.tile_pool(name="pw", bufs=1) as pw, \
             tc.tile_pool(name="pfw", bufs=1) as pfw:
            wsg_sb = pw.tile([P, HC * ISC], F16)
            wsu_sb = pw.tile([P, HC * ISC], F16)
            wsd_sb = [pw.tile([P, H], F16, tag=f"wsd{sp}", name=f"wsd{sp}")
                      for sp in range(2)]
            gwr = pw.tile([P, HC * E], F32)
            gate_b_sb = pw.tile([P, E], F32)
            ut_sb = pw.tile([P, P], F16)
            bc127 = pw.tile([P, P], F16)
            slb_sb = pw.tile([8, TB * P], F16)
            em = [pw.tile([P, E], F32, tag=f"em{e}", name=f"em{e}")
                  for e in range(2)]

            NJ = IP // 2  # 4 hc rows per 1MB pair load
            wseq = [(k, ei, j) for ei in range(2) for k in ("g", "u")
                    for j in range(NJ)]
            wring = {}
            WIN = 3

            def issue_pair(i):
                k, ei, j = wseq[i]
                src = ex["we_g"] if k == "g" else ex["we_u"]
                t_ = pfw.tile([P, 4 * I], F16, tag="wp", bufs=WIN,
                              name=f"wp{i}")
                eng = nc.sync if i % 2 == 0 else nc.scalar
                eng.dma_start(
                    t_[:].rearrange("p (four i) -> p four i", four=4),
                    src[ei, j * 4 * P:(j + 1) * 4 * P, :].rearrange(
                        "(four a) i -> a four i", a=P))
                wring[i] = t_

            # ======== Phase A: norm + transpose + QKV + rope ========
            with tc.tile_pool(name="pab", bufs=1) as pab:
                qT = [pab.tile([P, T], F16, tag=f"qT{j}", name=f"qT{j}")
                      for j in range(2)]
                kT = pab.tile([P, T], F16)
                v_tm = pab.tile([P, TB * D], F16)
                wo_sb = [pab.tile([P, H], F16, tag=f"wo{j}", name=f"wo{j}")
                         for j in range(2)]
                diagm = pab.tile([P, P], F16)

                with tc.tile_pool(name="pa", bufs=1) as pa, \
                     tc.tile_pool(name="pa2", bufs=2) as pa2:
                    hidbs = []
                    for b in range(TB):
                        t_ = pa2.tile([P, H], F16, tag="hidb", bufs=8,
                                      name=f"hidb{b}")
                        nc.sync.dma_start(t_[:], ex["hid"][b * P:(b + 1) * P, :])
                        hidbs.append(t_)
                    nc.sync.dma_start(wo_sb[0][:], ex["wo0"][:])
                    nc.sync.dma_start(wo_sb[1][:], ex["wo1"][:])
                    nc.sync.dma_start(diagm[:], ex["diagmask"][:])
                    cosq = pa.tile([D, T], F16)
                    sinq = pa.tile([D, T], F16)
                    cosk = pa.tile([D, T], F16)
                    sink = pa.tile([D, T], F16)
                    for t_, s_ in [(cosq, "cosq"), (sinq, "sinq"),
                                   (cosk, "cosk"), (sink, "sink")]:
                        nc.scalar.dma_start(t_[:], ex[s_][:])
                    permh = pa.tile([P, P], F16)
                    nc.scalar.dma_start(permh[:], ex["permh"][:])
                    wqkv_sb = pa.tile([P, HC * 512], F16)
                    nc.sync.dma_start(wqkv_sb[:], ex["w_qkv_pk"][:])
                    # persistent-weight prefetch (runs during attention)
                    nc.scalar.dma_start(wsg_sb[:], ex["ws_g_pk"][:])
                    nc.scalar.dma_start(wsu_sb[:], ex["ws_u_pk"][:])
                    for sp in range(2):
                        nc.scalar.dma_start(wsd_sb[sp][:],
                                            ex["ws_d"][sp * P:(sp + 1) * P, :])
                    nc.scalar.dma_start(gwr[:], ex["gate_w_pk"][:])
                    nc.scalar.dma_start(gate_b_sb[:], ex["gate_b"][:])
                    nc.scalar.dma_start(ut_sb[:], ex["ut_in"][:])
                    nc.scalar.dma_start(bc127[:], ex["bcast127"][:])
                    nc.scalar.dma_start(slb_sb[:], ex["slb_in"][:])
                    nc.scalar.dma_start(em[0][:], ex["emask0"][:])
                    nc.scalar.dma_start(em[1][:], ex["emask1"][:])
                    for i in range(2):
                        issue_pair(i)

                    x0T = [pa.tile([P, T], F16, tag=f"x0T{hc}",
                                   name=f"x0T{hc}") for hc in range(HC)]
                    qraw = [pa.tile([P, T], F16, tag=f"qraw{j}",
                                    name=f"qraw{j}") for j in range(2)]
                    kraw = pa.tile([P, T], F16)
                    vraw = pa.tile([P, T], F16)
                    dump = pa.tile([P, H], F32)

                    with tc.tile_pool(name="psA1", bufs=2, space="PSUM") as psA1, \
                         tc.tile_pool(name="psA2", bufs=1, space="PSUM") as psA2:
                        for n in range(2):
                            x0hs = []
                            for bb in range(TB // 2):
                                b = n * (TB // 2) + bb
                                hidb = hidbs[b]
                                ssum = pa2.tile([P, 1], F32, tag="ssum")
                                nc.scalar.activation(dump[:], hidb[:],
                                                     AF.Square,
                                                     accum_out=ssum[:, :1])
                                rms = pa2.tile([P, 1], F32, tag="rms")
                                nc.scalar.activation(rms[:], ssum[:], AF.Sqrt,
                                                     bias=eps_t[:, :1],
                                                     scale=1.0 / H)
                                inv = pa2.tile([P, 1], F32, tag="inv")
                                nc.vector.reciprocal(inv[:], rms[:])
                                x0h = pa2.tile([P, H], F16, tag="x0h", bufs=4,
                                               name=f"x0h{b}")
                                nc.vector.tensor_scalar_mul(x0h[:], hidb[:],
                                                            inv[:, :1])
                                x0hs.append(x0h)
                            sl = slice(n * 512, (n + 1) * 512)
                            for hc in range(HC):
                                tp = psA1.tile([P, 512], F16, tag="tpA")
                                for bb in range(4):
                                    nc.tensor.transpose(
                                        tp[:, bb * P:(bb + 1) * P],
                                        x0hs[bb][:, hc * P:(hc + 1) * P],
                                        identh[:])
                                if hc % 2 == 0:
                                    nc.vector.tensor_copy(x0T[hc][:, sl], tp[:])
                                else:
                                    nc.scalar.activation(x0T[hc][:, sl], tp[:],
                                                         AF.Copy)
                            ps4 = [psA2.tile([P, 512], F32, tag=f"qkv{j}",
                                             name=f"qkv{j}_{n}")
                                   for j in range(4)]
                            for hc in range(HC):
                                for j, c0 in enumerate([0, 128, 256, 384]):
                                    nc.tensor.matmul(
                                        ps4[j][:],
                                        wqkv_sb[:, hc * 512 + c0:
                                                hc * 512 + c0 + P],
                                        x0T[hc][:, sl],
                                        start=(hc == 0), stop=(hc == HC - 1))
                            for j, dst in enumerate([qraw[0], qraw[1],
                                                     kraw, vraw]):
                                if j % 2 == 0:
                                    nc.vector.tensor_copy(dst[:, sl], ps4[j][:])
                                else:
                                    nc.scalar.activation(dst[:, sl], ps4[j][:],
                                                         AF.Copy)

                    with tc.tile_pool(name="psA3", bufs=2, space="PSUM") as psA3, \
                         tc.tile_pool(name="psA4", bufs=2, space="PSUM") as psA4:
                        for src, dst, c_, s_ in [(qraw[0], qT[0], cosq, sinq),
                                                 (qraw[1], qT[1], cosq, sinq),
                                                 (kraw, kT, cosk, sink)]:
                            sw = psA3.tile([P, T], F32, tag="sw")
                            for nn in range(2):
                                sl = slice(nn * 512, (nn + 1) * 512)
                                nc.tensor.matmul(sw[:, sl], permh[:], src[:, sl],
                                                 start=True, stop=True)
                            t1 = pa2.tile([P, T], F16, tag="ropet1")
                            nc.gpsimd.tensor_mul(t1[:], src[:], c_[:])
                            t2 = pa2.tile([P, T], F16, tag="ropet2")
                            nc.vector.tensor_mul(t2[:], sw[:], s_[:])
                            nc.gpsimd.tensor_add(dst[:], t1[:], t2[:])
                        for g4 in range(2):
                            tp = psA4.tile([P, 512], F16, tag="tpV")
                            for bb in range(4):
                                b = g4 * 4 + bb
                                nc.tensor.transpose(
                                    tp[:, bb * P:(bb + 1) * P],
                                    vraw[:, b * P:(b + 1) * P], identh[:])
                            nc.vector.tensor_copy(
                                v_tm[:, g4 * 512:(g4 + 1) * 512], tp[:])

                # ==== Phase B: per-block attention + o_proj + chunked RS1 ==
                with tc.tile_pool(name="pb", bufs=1) as pb, \
                     tc.tile_pool(name="pb2", bufs=2) as pb2:
                    atn = [pb.tile([P, TB * P], F16, tag=f"atn{h}",
                                   name=f"atn{h}") for h in range(2)]
                    with tc.tile_pool(name="psBs", bufs=2, space="PSUM") as psBs, \
                         tc.tile_pool(name="psBt", bufs=1, space="PSUM") as psBt, \
                         tc.tile_pool(name="psAv", bufs=1, space="PSUM") as psAv, \
                         tc.tile_pool(name="psBp", bufs=1, space="PSUM") as psBp:
                        for qc in range(TB):
                            W = (qc + 1) * P
                            probs_h = []
                            for h in range(2):
                                sc = psBs.tile([P, T], F32, tag="sc")
                                for c0 in range(0, W, 512):
                                    c1 = min(c0 + 512, W)
                                    nc.tensor.matmul(
                                        sc[:, c0:c1],
                                        qT[h][:, qc * P:(qc + 1) * P],
                                        kT[:, c0:c1], start=True, stop=True)
                                nc.vector.tensor_tensor(
                                    out=sc[:, W - P:W], in0=sc[:, W - P:W],
                                    in1=diagm[:], op=ALU.add)
                                probs = pb2.tile([P, T], F16, tag="probs",
                                                 bufs=4)
                                ssum = pb2.tile([P, 1], F32, tag="esum")
                                nc.scalar.activation(probs[:, :W], sc[:, :W],
                                                     AF.Exp,
                                                     accum_out=ssum[:, :1])
                                rec = pb2.tile([P, 1], F32, tag="rec")
                                nc.vector.reciprocal(rec[:], ssum[:])
                                nc.vector.tensor_scalar_mul(probs[:, :W],
                                                            probs[:, :W],
                                                            rec[:, :1])
                                probs_h.append(probs)
                            oTb = []
                            av = psAv.tile([P, 2 * P], F32, tag="av",
                                           name=f"av_{qc}")
                            for h in range(2):
                                probs = probs_h[h]
                                for g4 in range(0, qc + 1, 4):
                                    cnt = min(4, qc + 1 - g4)
                                    tp = psBt.tile([P, 512], F16, tag="tpB")
                                    for i in range(cnt):
                                        kc = g4 + i
                                        nc.tensor.transpose(
                                            tp[:, i * P:(i + 1) * P],
                                            probs[:, kc * P:(kc + 1) * P],
                                            identh[:])
                                    dst = atn[h][:, g4 * P:(g4 + cnt) * P]
                                    if (qc + h) % 2 == 0:
                                        nc.vector.tensor_copy(dst,
                                                              tp[:, :cnt * P])
                                    else:
                                        nc.scalar.activation(dst,
                                                             tp[:, :cnt * P],
                                                             AF.Copy)
                                for kc in range(qc + 1):
                                    nc.tensor.matmul(
                                        av[:, h * P:(h + 1) * P],
                                        v_tm[:, kc * P:(kc + 1) * P],
                                        atn[h][:, kc * P:(kc + 1) * P],
                                        start=(kc == 0), stop=(kc == qc))
                                ot = pb2.tile([P, P], F16, tag=f"oTb{h}")
                                if h == 0:
                                    nc.vector.tensor_copy(
                                        ot[:], av[:, h * P:(h + 1) * P])
                                else:
                                    nc.scalar.activation(
                                        ot[:], av[:, h * P:(h + 1) * P],
                                        AF.Copy)
                                oTb.append(ot)
                            q_ = qc // 4
                            ro = (qc % 4) * P
                            ob = pb2.tile([P, H], F16, tag="ob")
                            for nn in range(2):
                                ps = psBp.tile([P, 1024], F32, tag="psO")
                                for h in range(2):
                                    for q2 in range(2):
                                        s2 = slice(q2 * 512, (q2 + 1) * 512)
                                        nc.tensor.matmul(
                                            ps[:, s2], oTb[h][:],
                                            wo_sb[h][:, nn * 1024 + q2 * 512:
                                                      nn * 1024 + (q2 + 1) * 512],
                                            start=(h == 0), stop=(h == 1))
                                dst = ob[:, nn * 1024:(nn + 1) * 1024]
                                if nn == 0:
                                    nc.vector.tensor_copy(dst, ps[:])
                                else:
                                    nc.scalar.activation(dst, ps[:], AF.Copy)
                            eng = nc.sync if qc % 2 == 0 else nc.scalar
                            eng.dma_start(rs1_in[q_][ro:ro + P, :], ob[:])
                            if qc % 4 == 3:
                                nc.gpsimd.collective_compute(
                                    "ReduceScatter", ALU.add,
                                    ins=[rs1_in[q_].opt()],
                                    outs=[rs1_out[q_].opt()],
                                    replica_groups=RG)

            # ======== Phase D: residual + norm + local router ========
            with tc.tile_pool(name="pd", bufs=1) as pd:
                hid_sl = pd.tile([P, H], F32)
                nc.sync.dma_start(hid_sl[:], ex["hid_slice"][:])
                attn_sl = pd.tile([P, H], F16)
                for q in range(NCH):
                    nc.sync.dma_start(attn_sl[q * SH:(q + 1) * SH, :],
                                      rs1_out[q][:])
                res_sb = pd.tile([P, H], F32)
                nc.vector.tensor_add(res_sb[:], hid_sl[:], attn_sl[:])
                nc.sync.dma_start(res_slice[:], res_sb[:])
                dump2 = pd.tile([P, H], F32)
                ssum = pd.tile([P, 1], F32)
                nc.scalar.activation(dump2[:], res_sb[:], AF.Square,
                                     accum_out=ssum[:, :1])
                rms = pd.tile([P, 1], F32)
                nc.scalar.activation(rms[:], ssum[:], AF.Sqrt,
                                     bias=eps_t[:, :1], scale=1.0 / H)
                inv = pd.tile([P, 1], F32)
                nc.vector.reciprocal(inv[:], rms[:])
                x_sl = pd.tile([P, H], F32)
                nc.vector.tensor_scalar_mul(x_sl[:], res_sb[:], inv[:, :1])
                payx = pd.tile([P, H], F16)
                nc.vector.tensor_copy(payx[:], x_sl[:])
                nc.scalar.dma_start(agx1_in[:, 0:H // 2], payx[:, 0:H // 2])
                nc.sync.dma_start(agx2_in[:], payx[:, H // 2:H])
                # local router on fp32 x
                xsT = pd.tile([P, HC * P], F32)
                with tc.tile_pool(name="psDt", bufs=2, space="PSUM") as psDt:
                    for g4 in range(4):
                        tp = psDt.tile([P, 512], F32, tag="tpD")
                        for i in range(4):
                            hc = g4 * 4 + i
                            nc.tensor.transpose(
                                tp[:, i * P:(i + 1) * P],
                                x_sl[:, hc * P:(hc + 1) * P], identf[:])
                        nc.vector.tensor_copy(
                            xsT[:, g4 * 512:(g4 + 1) * 512], tp[:])
                with tc.tile_pool(name="psDr", bufs=1, space="PSUM") as psDr:
                    lg = psDr.tile([P, E], F32, tag="lg")
                    for hc in range(HC):
                        nc.tensor.matmul(lg[:], xsT[:, hc * P:(hc + 1) * P],
                                         gwr[:, hc * E:(hc + 1) * E],
                                         start=(hc == 0), stop=(hc == HC - 1))
                    sig = pd.tile([P, E], F32)
                    nc.scalar.activation(sig[:], lg[:], AF.Sigmoid)
                sb_ = pd.tile([P, E], F32)
                nc.vector.tensor_add(sb_[:], sig[:], gate_b_sb[:])
                mx8 = pd.tile([P, 8], F32)
                nc.vector.max(out=mx8[:], in_=sb_[:])
                s1 = pd.tile([P, E], F32)
                nc.vector.tensor_tensor(out=s1[:], in0=sb_[:],
                                        in1=mx8[:, 0:1].to_broadcast([P, E]),
                                        op=ALU.is_equal)
                s2 = pd.tile([P, E], F32)
                nc.vector.tensor_tensor(out=s2[:], in0=sb_[:],
                                        in1=mx8[:, 1:2].to_broadcast([P, E]),
                                        op=ALU.is_equal)
                nc.vector.tensor_add(s1[:], s1[:], s2[:])
                nc.vector.tensor_scalar_min(s1[:], s1[:], 1.0)
                wa = pd.tile([P, E], F32)
                nc.vector.tensor_mul(wa[:], s1[:], sig[:])
                nrm = pd.tile([P, 1], F32)
                nc.vector.reduce_sum(nrm[:], wa[:], axis=AX.X)
                rec = pd.tile([P, 1], F32)
                nc.vector.reciprocal(rec[:], nrm[:])
                paw = pd.tile([P, WP], F16)
                nc.vector.tensor_scalar_mul(paw[:, 0:E], wa[:], rec[:, :1])
                selh = pd.tile([P, E], F16)
                nc.vector.tensor_copy(selh[:], s1[:])
                uml = pd.tile([P, E], F32)
                nc.vector.tensor_scalar(out=uml[:], in0=selh[:],
                                        scalar1=-BIG2, scalar2=BIG2,
                                        op0=ALU.mult, op1=ALU.add)
                pre_l = pd.tile([P, E], F16)
                with tc.tile_pool(name="psDp", bufs=1, space="PSUM") as psDp:
                    prp = psDp.tile([P, E], F32, tag="prp")
                    nc.tensor.matmul(prp[:], ut_sb[:], selh[:],
                                     start=True, stop=True)
                    nc.vector.tensor_copy(pre_l[:], prp[:])
                    nc.vector.tensor_tensor(out=paw[:, E:2 * E], in0=prp[:],
                                            in1=uml[:], op=ALU.add)
                    tbp = psDp.tile([P, E], F32, tag="tbp")
                    nc.tensor.matmul(tbp[:], bc127[:], pre_l[:],
                                     start=True, stop=True)
                    nc.vector.tensor_copy(paw[:, 2 * E:3 * E], tbp[:])
                nc.scalar.dma_start(agx1_in[:, H // 2:H // 2 + WP], paw[:])

            nc.gpsimd.collective_compute(
                "AllGather", ALU.bypass, ins=[agx1_in.opt()],
                outs=[x_tmA.opt()], replica_groups=RG)
            nc.gpsimd.collective_compute(
                "AllGather", ALU.bypass, ins=[agx2_in.opt()],
                outs=[x_tmB.opt()], replica_groups=RG)

            # ======== Phase X: token lists + x^T + shared + experts ======
            with tc.tile_pool(name="pg", bufs=1) as pg, \
                 tc.tile_pool(name="pg2", bufs=2) as pg2:
                xT = [pg.tile([P, T], F16, tag=f"xT{hc}", name=f"xT{hc}")
                      for hc in range(HC)]
                totals = pg.tile([8, E], F16)
                grank = [pg.tile([P, E], F32, tag=f"grank{b}",
                                 name=f"grank{b}") for b in range(TB)]
                wb0 = H // 2
                with tc.tile_pool(name="psXr", bufs=2, space="PSUM") as psXr:
                    nc.sync.dma_start(
                        totals[:].rearrange("b (o e) -> b o e", o=1),
                        x_tmA[:].rearrange("(b p) e -> b p e", p=P)[
                            :, 0:1, wb0 + 2 * E:wb0 + 3 * E])
                    wrbs = []
                    for b in range(TB):
                        wrb = pg2.tile([P, WP], F16, tag="wrb", bufs=8,
                                       name=f"wrb{b}")
                        nc.sync.dma_start(
                            wrb[:], x_tmA[b * P:(b + 1) * P, wb0:wb0 + WP])
                        wrbs.append(wrb)
                    for b in range(TB):
                        ofs = psXr.tile([P, E], F32, tag="ofs")
                        nc.tensor.matmul(ofs[:], slb_sb[:, b * P:(b + 1) * P],
                                         totals[:], start=True, stop=True)
                        nc.vector.tensor_tensor(out=grank[b][:],
                                                in0=wrbs[b][:, E:2 * E],
                                                in1=ofs[:], op=ALU.add)
                        gm = pg2.tile([P, E], F32, tag="gm")
                        nc.vector.tensor_scalar(out=gm[:], in0=grank[b][:],
                                                scalar1=float(CAP),
                                                scalar2=BIG,
                                                op0=ALU.is_gt, op1=ALU.mult)
                        nc.vector.tensor_add(grank[b][:], grank[b][:], gm[:])
                    sent = pg.tile([P, 1], I32)
                    nc.vector.memset(sent[:], 1000000)
                    for kk in range(2 * SL // P):
                        nc.sync.dma_start(tok_lists[kk * P:(kk + 1) * P, :],
                                          sent[:])
                    for b in range(TB):
                        tok = pg2.tile([P, 1], I32, tag="tok")
                        nc.gpsimd.iota(tok[:], pattern=[[0, 1]], base=b * P,
                                       channel_multiplier=1)
                        for ei in range(2):
                            gsel = pg2.tile([P, E], F32, tag="gsel")
                            nc.vector.tensor_mul(gsel[:], grank[b][:],
                                                 em[ei][:])
                            ridx = pg2.tile([P, 1], F32, tag="ridx")
                            nc.vector.reduce_sum(ridx[:], gsel[:], axis=AX.X)
                            nc.vector.tensor_scalar_add(ridx[:], ridx[:],
                                                        float(ei * SL - 1))
                            ridx_i = pg2.tile([P, 1], I32, tag="ridxi")
                            nc.vector.tensor_copy(ridx_i[:], ridx[:])
                            nc.gpsimd.indirect_dma_start(
                                out=tok_lists[:],
                                out_offset=bass.IndirectOffsetOnAxis(
                                    ap=ridx_i[:, :1], axis=0),
                                in_=tok[:], in_offset=None,
                                bounds_check=2 * SL - 1, oob_is_err=False)

                # x^T build: half A (hc 0-7) then half B (hc 8-15),
                # shared-expert gate pass interleaved
                hsh = [pg.tile([P, T], F16, tag=f"hs{sp}", name=f"hs{sp}")
                       for sp in range(2)]
                gsh = [pg.tile([P, T], F16, tag=f"gsh{sp}", name=f"gsh{sp}")
                       for sp in range(2)]
                with tc.tile_pool(name="psXt", bufs=2, space="PSUM") as psXt, \
                     tc.tile_pool(name="psS", bufs=1, space="PSUM") as psS:
                    gps = [psS.tile([P, T], F32, tag=f"sgp{sp}",
                                    name=f"sgp{sp}") for sp in range(2)]
                    for half, src_tm, hclo in [(0, x_tmA, 0), (1, x_tmB, 8)]:
                        for n in range(2):
                            xbs = []
                            for bb in range(4):
                                b = n * 4 + bb
                                xb = pg2.tile([P, H // 2], F16, tag="xb",
                                              bufs=4, name=f"xb{half}_{b}")
                                eng = nc.sync if bb % 2 == 0 else nc.scalar
                                eng.dma_start(
                                    xb[:], src_tm[b * P:(b + 1) * P,
                                                  0:H // 2])
                                xbs.append(xb)
                            sl = slice(n * 512, (n + 1) * 512)
                            for hc8 in range(8):
                                hc = hclo + hc8
                                tp = psXt.tile([P, 512], F16, tag="tpX")
                                for bb in range(4):
                                    nc.tensor.transpose(
                                        tp[:, bb * P:(bb + 1) * P],
                                        xbs[bb][:, hc8 * P:(hc8 + 1) * P],
                                        identh[:])
                                if hc % 2 == 0:
                                    nc.vector.tensor_copy(xT[hc][:, sl], tp[:])
                                else:
                                    nc.scalar.activation(xT[hc][:, sl], tp[:],
                                                         AF.Copy)
                        # shared gate pass for this half's hc range
                        for hc8 in range(8):
                            hc = hclo + hc8
                            for sp in range(2):
                                c0 = hc * ISC + sp * P
                                for nn in range(2):
                                    sl = slice(nn * 512, (nn + 1) * 512)
                                    nc.tensor.matmul(gps[sp][:, sl],
                                                     wsg_sb[:, c0:c0 + P],
                                                     xT[hc][:, sl],
                                                     start=(hc == 0),
                                                     stop=(hc == HC - 1))
                    for sp in range(2):
                        nc.vector.tensor_copy(gsh[sp][:], gps[sp][:])
                        nc.scalar.activation(gsh[sp][:], gsh[sp][:], AF.Silu)
                    ups = [psS.tile([P, T], F32, tag=f"sgp{sp}",
                                    name=f"sup{sp}") for sp in range(2)]
                    for hc in range(HC):
                        for sp in range(2):
                            c0 = hc * ISC + sp * P
                            for nn in range(2):
                                sl = slice(nn * 512, (nn + 1) * 512)
                                nc.tensor.matmul(ups[sp][:, sl],
                                                 wsu_sb[:, c0:c0 + P],
                                                 xT[hc][:, sl],
                                                 start=(hc == 0),
                                                 stop=(hc == HC - 1))
                    for sp in range(2):
                        nc.vector.tensor_mul(hsh[sp][:], gsh[sp][:],
                                             ups[sp][:])
                with tc.tile_pool(name="psS3", bufs=2, space="PSUM") as psS3:
                    for nn in range(2):
                        for tb2 in range(TB // 2):
                            sd = pg2.tile([P, 2048], F16, tag="sd", bufs=1)
                            for two in range(2):
                                tb_ = tb2 * 2 + two
                                ps3 = psS3.tile([P, 1024], F32, tag="psSd")
                                for sp in range(2):
                                    for q2 in range(2):
                                        s2 = slice(q2 * 512, (q2 + 1) * 512)
                                        nc.tensor.matmul(
                                            ps3[:, s2],
                                            hsh[sp][:, tb_ * P:(tb_ + 1) * P],
                                            wsd_sb[sp][:, nn * 1024 + q2 * 512:
                                                        nn * 1024 + (q2 + 1) * 512],
                                            start=(sp == 0), stop=(sp == 1))
                                dst = sd[:, two * 1024:(two + 1) * 1024]
                                if two == 0:
                                    nc.vector.tensor_copy(dst, ps3[:])
                                else:
                                    nc.scalar.activation(dst, ps3[:], AF.Copy)
                            eng = nc.sync if tb2 % 2 == 0 else nc.scalar
                            eng.dma_start(
                                rs2_in[nn][tb2 * 2 * P:(tb2 + 1) * 2 * P, :]
                                .rearrange("(two p) c -> p two c", two=2),
                                sd[:].rearrange("p (two c) -> p two c", two=2))

                # ======== Expert gathers + wcol (gpsimd ahead of PE) ======
                KL = [P, CAP - P]
                idx_sb = [[pg.tile([P if k == 0 else CAP - P, 1], I32,
                                   tag=f"idx{ei}_{k}",
                                   name=f"idx{ei}_{k}") for k in range(2)]
                          for ei in range(2)]
                gxT = [pg.tile([P, HC * SL], F16, tag=f"gxT{ei}",
                               name=f"gxT{ei}") for ei in range(2)]
                wcol = [[pg.tile([P if k == 0 else CAP - P, 1], F32,
                                 tag=f"wcol{ei}_{k}",
                                 name=f"wcol{ei}_{k}") for k in range(2)]
                        for ei in range(2)]
                with tc.tile_pool(name="psFt", bufs=2, space="PSUM") as psFt:
                    for ei in range(2):
                        gxA = [None, None]
                        gxB = [None, None]
                        for k in range(2):
                            nc.sync.dma_start(
                                idx_sb[ei][k][:],
                                tok_lists[ei * SL + k * P:
                                          ei * SL + k * P + KL[k], :])
                            ga_ = pg2.tile([P, H // 2 + WP], F16, tag="gxA",
                                           name=f"gxA{ei}_{k}")
                            nc.vector.memset(ga_[:KL[k], :], 0.0)
                            nc.gpsimd.indirect_dma_start(
                                out=ga_[:KL[k], :], out_offset=None,
                                in_=x_tmA[:],
                                in_offset=bass.IndirectOffsetOnAxis(
                                    ap=idx_sb[ei][k][:, :1], axis=0),
                                bounds_check=T - 1, oob_is_err=False)
                            gb_ = pg2.tile([P, H // 2], F16, tag="gxB",
                                           name=f"gxB{ei}_{k}")
                            nc.vector.memset(gb_[:KL[k], :], 0.0)
                            nc.gpsimd.indirect_dma_start(
                                out=gb_[:KL[k], :], out_offset=None,
                                in_=x_tmB[:],
                                in_offset=bass.IndirectOffsetOnAxis(
                                    ap=idx_sb[ei][k][:, :1], axis=0),
                                bounds_check=T - 1, oob_is_err=False)
                            gxA[k] = ga_
                            gxB[k] = gb_
                        # wcol first (vector-only, unblocks nothing behind)
                        for k in range(2):
                            wtmp_f = pg2.tile([P, E], F32, tag="wtmp")
                            wtmp = wtmp_f[:KL[k], :]
                            nc.vector.tensor_mul(
                                wtmp, gxA[k][:KL[k], wb0:wb0 + E],
                                em[ei][:KL[k], :])
                            nc.vector.reduce_sum(wcol[ei][k][:], wtmp,
                                                 axis=AX.X)
                        for hp in range(HC // 2):
                            tp = psFt.tile([P, 2 * CAP], F16, tag="tpF")
                            for i in range(2):
                                hc = hp * 2 + i
                                o0 = i * CAP
                                gsrc = gxA if hc < 8 else gxB
                                c0 = (hc % 8) * P
                                nc.tensor.transpose(
                                    tp[:, o0:o0 + P],
                                    gsrc[0][:, c0:c0 + P],
                                    identh[:])
                                nc.tensor.transpose(
                                    tp[:, o0 + P:o0 + CAP],
                                    gsrc[1][:KL[1], c0:c0 + P],
                                    identh[:KL[1], :KL[1]])
                            dst = gxT[ei][:].rearrange(
                                "p (hc c) -> p hc c", hc=HC)[
                                :, hp * 2:hp * 2 + 2, 0:CAP]
                            src = tp[:].rearrange("p (hc c) -> p hc c", hc=2)
                            if hp % 2 == 0:
                                nc.vector.tensor_copy(dst, src)
                            else:
                                nc.scalar.activation(dst, src, AF.Copy)

                # ======== Experts: I-partitioned gate/up ======
                h_sb = [pg.tile([P, IP * CAP], F16, tag=f"h_sb{ei}",
                                name=f"h_sb{ei}") for ei in range(2)]
                sg_sb = pg.tile([P, IP * CAP], F16)
                wd_res = [pg.tile([P, H], F16, tag=f"wd{e}_{ip}",
                                  name=f"wd{e}_{ip}")
                          for e in range(2) for ip in range(IP)]
                for ip in range(IP):
                    eng = nc.sync if ip % 2 == 0 else nc.scalar
                    eng.dma_start(wd_res[ip][:],
                                  ex["we_d"][0, ip * P:(ip + 1) * P, :])
                wi = 2
                for ei in range(2):
                    if ei == 1:
                        for ip in range(IP):
                            eng = nc.sync if ip % 2 == 0 else nc.scalar
                            eng.dma_start(
                                wd_res[IP + ip][:],
                                ex["we_d"][1, ip * P:(ip + 1) * P, :])
                    with tc.tile_pool(name=f"psF1{ei}", bufs=1,
                                      space="PSUM") as psF1:
                        acc = [psF1.tile([P, 256], F32, tag=f"acc{ip}",
                                         name=f"acc{ip}_{ei}")
                               for ip in range(IP)]
                        for kind in ("g", "u"):
                            base = (0 if kind == "g" else NJ) + ei * 2 * NJ
                            for j in range(NJ):
                                wp = wring[base + j]
                                if wi < len(wseq):
                                    issue_pair(wi)
                                    wi += 1
                                for four in range(4):
                                    hc = 4 * j + four
                                    for ip in range(IP):
                                        nc.tensor.matmul(
                                            acc[ip][:, :CAP],
                                            wp[:, four * I + ip * P:
                                               four * I + (ip + 1) * P],
                                            gxT[ei][:, hc * SL:hc * SL + CAP],
                                            start=(hc == 0),
                                            stop=(hc == HC - 1))
                            if kind == "g":
                                for ip in range(IP):
                                    nc.scalar.activation(
                                        sg_sb[:, ip * CAP:(ip + 1) * CAP],
                                        acc[ip][:, :CAP], AF.Silu)
                            else:
                                for ip in range(IP):
                                    nc.vector.tensor_mul(
                                        h_sb[ei][:, ip * CAP:(ip + 1) * CAP],
                                        sg_sb[:, ip * CAP:(ip + 1) * CAP],
                                        acc[ip][:, :CAP])

                # ======== Down-projections, column-half outer + RS2 ======
                with tc.tile_pool(name="psF3", bufs=2, space="PSUM") as psF3:
                    for nn in range(2):
                        for ei in range(2):
                            for k in range(2):
                                kl = KL[k]
                                koff = k * P
                                psd = psF3.tile([P, 1024], F32, tag="fd")
                                for ip in range(IP):
                                    c0 = ip * CAP + koff
                                    for q2 in range(2):
                                        s2 = slice(q2 * 512, (q2 + 1) * 512)
                                        nc.tensor.matmul(
                                            psd[:kl, s2],
                                            h_sb[ei][:, c0:c0 + kl],
                                            wd_res[ei * IP + ip][
                                                :, nn * 1024 + q2 * 512:
                                                nn * 1024 + (q2 + 1) * 512],
                                            start=(ip == 0),
                                            stop=(ip == IP - 1))
                                out_f = pg2.tile([P, 1024], F16, tag="outsb",
                                                 name=f"outsb{nn}{ei}{k}")
                                nc.vector.tensor_scalar_mul(
                                    out_f[:kl, :], psd[:kl, :],
                                    wcol[ei][k][:, :1])
                                nc.gpsimd.indirect_dma_start(
                                    out=rs2_in[nn][:],
                                    out_offset=bass.IndirectOffsetOnAxis(
                                        ap=idx_sb[ei][k][:, :1], axis=0),
                                    in_=out_f[:kl, :], in_offset=None,
                                    bounds_check=T - 1, oob_is_err=False,
                                    compute_op=ALU.add)
                        nc.gpsimd.collective_compute(
                            "ReduceScatter", ALU.add, ins=[rs2_in[nn].opt()],
                            outs=[rs2_out[nn].opt()], replica_groups=RG)

            for nn in range(2):
                nc.sync.dma_start(out_slice[:, nn * 1024:(nn + 1) * 1024],
                                  rs2_out[nn][:])


_CACHE = {}


def _build():
    key = "nc"
    if key in _CACHE:
        return _CACHE[key]
    nc = bacc.Bacc("TRN2", target_bir_lowering=False, debug=False,
                   num_devices=NCN)
    with tile.TileContext(nc) as tc:
        _emit(nc, tc)
    nc.compile()
    _CACHE[key] = nc
    return nc


def _perm_rows(c):
    return np.concatenate([np.arange(q * CH + c * SH, q * CH + (c + 1) * SH)
                           for q in range(NCH)])


def _host_prep(inputs):
    f16 = np.float16
    pos = np.asarray(inputs["positions"]).astype(np.float64)
    hid = np.asarray(inputs["hidden_states"], np.float32)
    w_in = np.asarray(inputs["w_in_ln"], np.float32)
    w_post = np.asarray(inputs["w_post_ln"], np.float32)
    wq = np.asarray(inputs["wq"], np.float32) * w_in[:, None]
    wk = np.asarray(inputs["wk"], np.float32) * w_in[:, None]
    wv = np.asarray(inputs["wv"], np.float32) * w_in[:, None]
    wo = np.asarray(inputs["wo"], np.float32)
    gate_w = np.asarray(inputs["gate_w"], np.float32) * w_post[None, :]
    gate_b = np.asarray(inputs["gate_bias"], np.float32).reshape(1, E)
    we_g = (np.asarray(inputs["we_gate"], np.float32)
            * w_post[None, :, None]).astype(f16)
    we_u = (np.asarray(inputs["we_up"], np.float32)
            * w_post[None, :, None]).astype(f16)
    we_d = np.asarray(inputs["we_down"], np.float32).astype(f16)
    ws_g = np.asarray(inputs["ws_gate"], np.float32) * w_post[:, None]
    ws_u = np.asarray(inputs["ws_up"], np.float32) * w_post[:, None]
    ws_d = np.asarray(inputs["ws_down"], np.float32).astype(f16)

    inv_freq = 1.0 / (THETA ** (np.arange(0, D, 2, dtype=np.float64) / D))
    f = pos[None, :] * inv_freq[:, None]
    cos2, sin2 = np.cos(f), np.sin(f)
    cosT = np.repeat(cos2, 2, axis=0).astype(np.float32)
    sinT = np.empty((D, T), np.float32)
    sinT[0::2] = -sin2
    sinT[1::2] = sin2
    s = 1.0 / np.sqrt(D)
    cosq, sinq = (cosT * s).astype(f16), (sinT * s).astype(f16)
    cosk, sink = cosT.astype(f16), sinT.astype(f16)

    ii = np.arange(P)
    diagmask = np.where(ii[:, None] >= ii[None, :], 0.0, NEG).astype(f16)
    ident = np.eye(P, dtype=np.float32)
    ut_in = np.triu(np.ones((P, P), np.float32)).astype(f16)
    slb_in = np.zeros((8, TB * P), np.float32)
    for b in range(TB):
        slb_in[:b, b * P:(b + 1) * P] = 1.0
    slb_in = slb_in.astype(f16)
    bc127 = np.zeros((P, P), np.float32)
    bc127[127, :] = 1.0
    bc127 = bc127.astype(f16)
    perm = np.zeros((P, P), np.float32)
    for i in range(0, P, 2):
        perm[i, i + 1] = 1.0
        perm[i + 1, i] = 1.0

    def pack_pk(w, width):  # w: [H, width]
        return np.ascontiguousarray(
            w.reshape(HC, P, width).transpose(1, 0, 2).reshape(P, HC * width))

    gate_w_pk = pack_pk(gate_w.T.astype(np.float32), E)

    maps = []
    for c in range(NCN):
        g = c // 2
        w_qkv = pack_pk(np.concatenate([
            wq[:, 2 * c * D:(2 * c + 1) * D],
            wq[:, (2 * c + 1) * D:(2 * c + 2) * D],
            wk[:, g * D:(g + 1) * D],
            wv[:, g * D:(g + 1) * D]], axis=1), 512).astype(f16)
        em0 = np.zeros((P, E), np.float32)
        em0[:, 2 * c] = 1.0
        em1 = np.zeros((P, E), np.float32)
        em1[:, 2 * c + 1] = 1.0
        maps.append({
            "hid": hid.astype(f16),
            "hid_slice": np.ascontiguousarray(hid[_perm_rows(c)]),
            "w_qkv_pk": w_qkv,
            "wo0": np.ascontiguousarray(wo[2 * c * D:(2 * c + 1) * D]).astype(f16),
            "wo1": np.ascontiguousarray(
                wo[(2 * c + 1) * D:(2 * c + 2) * D]).astype(f16),
            "cosq": cosq, "sinq": sinq, "cosk": cosk, "sink": sink,
            "permh": perm.astype(f16), "identh_in": ident.astype(f16),
            "identr_in": ident, "diagmask": diagmask,
            "gate_w_pk": gate_w_pk,
            "gate_b": np.broadcast_to(gate_b, (P, E)).astype(np.float32).copy(),
            "emask0": em0, "emask1": em1,
            "ut_in": ut_in, "slb_in": slb_in, "bcast127": bc127,
            "ws_g_pk": pack_pk(
                ws_g[:, c * ISC:(c + 1) * ISC].astype(np.float32), ISC
            ).astype(f16),
            "ws_u_pk": pack_pk(
                ws_u[:, c * ISC:(c + 1) * ISC].astype(np.float32), ISC
            ).astype(f16),
            "ws_d": np.ascontiguousarray(ws_d[c * ISC:(c + 1) * ISC]),
            "we_g": np.ascontiguousarray(we_g[2 * c:2 * c + 2]),
            "we_u": np.ascontiguousarray(we_u[2 * c:2 * c + 2]),
            "we_d": np.ascontiguousarray(we_d[2 * c:2 * c + 2]),
        })
    return maps


def kernel(trace=False, **inputs):
    nc = _build()
    maps = _host_prep(inputs)
    res = bass_utils.run_bass_kernel_spmd(
        nc, maps, core_ids=list(range(NCN)), trace=trace)
    out = np.empty((T, H), np.float32)
    resid = np.empty((T, H), np.float32)
    for c in range(NCN):
        rows = _perm_rows(c)
        out[rows] = res.results[c]["out_slice"].astype(np.float32)
        resid[rows] = res.results[c]["res_slice"]
    kernel.last_results = res
    return out, resid


# revision 25
# speedup vs baseline: 1.0154x; 1.0154x over previous
"""Ernie4 decoder layer (RMSNorm + GQA attention + shared expert + 16-expert
top-2 MoE) on 8 Trainium2 NeuronCores.

v3 (pipelined collectives):
  - Attention head-parallel, processed query-block-major: per 128-token
    block both heads' scores/softmax/AV and the o_proj run immediately,
    feeding 4 token-chunked ReduceScatters that fire DURING attention.
    Token ownership becomes permuted (32-row shards per chunk); the host
    permutes hid_slice in and unpermutes outputs.
  - AG splits in two column chunks: AGx1 (x lo-half) fires right after the
    norm; AGx2 (x hi-half | router payload) after the router. x^T build and
    the shared-expert gate pass consume the halves progressively.
  - Expert capacity 192; gate/up I-partitioned (no h transposes); expert
    weight-scale (wcol) gathers run before expert compute so the gpsimd
    queue never blocks the down-projections.
  - Down-proj weights load during expert gate/up; down-projections run
    column-half-outer feeding 2 chunked ReduceScatters so RS2a overlaps
    the second half's compute.
"""
import sys
sys.path.insert(0, "/opt/trn_rl_repo")

import numpy as np

import concourse.bass as bass
import concourse.bacc as bacc
import concourse.tile as tile
import concourse.mybir as mybir
from concourse import bass_utils

dt = mybir.dt
F32 = dt.float32
F16 = dt.float16
I32 = dt.int32
AF = mybir.ActivationFunctionType
ALU = mybir.AluOpType
AX = mybir.AxisListType

T, H, NH, NKV, D = 1024, 2048, 16, 4, 128
E, I, IS = 16, 1024, 2048
ISC = IS // 8
EPS = 1e-6
THETA = 10000.0
NCN = 8
P = 128
TB = T // P
HC = H // P
IP = I // P
CAP = 160               # per-expert compute capacity (rank mask)
SL = 256                # per-expert list-slot spacing (square layouts)
NCH = 2                 # RS1 token chunks
CH = T // NCH           # 256 tokens per chunk
SH = CH // NCN          # 32-row per-core shard per chunk
WP = 3 * E              # router payload width
BIG = 1.0e6
BIG2 = 30000.0
NEG = -30000.0
RG = [list(range(NCN))]


def _emit(nc, tc):
    ex = {}
    for name, shape, d in [
        ("hid", [T, H], F16), ("hid_slice", [P, H], F32),
        ("w_qkv_pk", [P, HC * 512], F16),
        ("wo0", [D, H], F16), ("wo1", [D, H], F16),
        ("cosq", [D, T], F16), ("sinq", [D, T], F16),
        ("cosk", [D, T], F16), ("sink", [D, T], F16),
        ("permh", [P, P], F16), ("identh_in", [P, P], F16),
        ("identr_in", [P, P], F32), ("diagmask", [P, P], F16),
        ("gate_w_pk", [P, HC * E], F32), ("gate_b", [P, E], F32),
        ("emask0", [P, E], F32), ("emask1", [P, E], F32),
        ("ut_in", [P, P], F16), ("slb_in", [8, TB * P], F16),
        ("bcast127", [P, P], F16),
        ("ws_g_pk", [P, HC * ISC], F16), ("ws_u_pk", [P, HC * ISC], F16),
        ("ws_d", [ISC, H], F16),
        ("we_g", [2, H, I], F16), ("we_u", [2, H, I], F16),
        ("we_d", [2, I, H], F16),
    ]:
        ex[name] = nc.dram_tensor(name, shape, d, kind="ExternalInput").ap()
    out_slice = nc.dram_tensor("out_slice", [P, H], F16, kind="ExternalOutput").ap()
    res_slice = nc.dram_tensor("res_slice", [P, H], F32, kind="ExternalOutput").ap()

    with tc.tile_pool(name="pp", bufs=1) as pp, \
         tc.tile_pool(name="dram", bufs=1, space="DRAM") as dram:
        rs1_in = [dram.tile([CH, H], F16, tag=f"rs1i{q}", name=f"rs1i{q}")
                  for q in range(NCH)]
        rs1_out = [dram.tile([SH, H], F16, tag=f"rs1o{q}", name=f"rs1o{q}")
                   for q in range(NCH)]
        agx1_in = dram.tile([P, H // 2 + WP], F16)
        agx2_in = dram.tile([P, H // 2], F16)
        x_tmA = dram.tile([T, H // 2 + WP], F16, addr_space="Shared")
        x_tmB = dram.tile([T, H // 2], F16, addr_space="Shared")
        tok_lists = dram.tile([2 * SL, 1], I32)
        rs2_in = [dram.tile([T, H // 2], F16, tag=f"rs2i{nn}",
                            name=f"rs2i{nn}") for nn in range(2)]
        rs2_out = [dram.tile([P, H // 2], F16, tag=f"rs2o{nn}",
                             name=f"rs2o{nn}") for nn in range(2)]

        identh = pp.tile([P, P], F16)
        nc.sync.dma_start(identh[:], ex["identh_in"][:])
        identf = pp.tile([P, P], F32)
        nc.sync.dma_start(identf[:], ex["identr_in"][:])
        eps_t = pp.tile([P, 1], F32)
        nc.vector.memset(eps_t[:], EPS)

        # ======== persistent weight pool (prefetched during attention) ====
        with tc.tile_pool(name="pw", bufs=1) as pw, \
             tc.tile_pool(name="pfw", bufs=1) as pfw:
            wsg_sb = pw.tile([P, HC * ISC], F16)
            wsu_sb = pw.tile([P, HC * ISC], F16)
            wsd_sb = [pw.tile([P, H], F16, tag=f"wsd{sp}", name=f"wsd{sp}")
                      for sp in range(2)]
            gwr = pw.tile([P, HC * E], F32)
            gate_b_sb = pw.tile([P, E], F32)
            ut_sb = pw.tile([P, P], F16)
            bc127 = pw.tile([P, P], F16)
            slb_sb = pw.tile([8, TB * P], F16)
            em = [pw.tile([P, E], F32, tag=f"em{e}", name=f"em{e}")
                  for e in range(2)]

            NJ = IP // 2  # 4 hc rows per 1MB pair load
            wseq = [(k, ei, j) for ei in range(2) for k in ("g", "u")
                    for j in range(NJ)]
            wring = {}
            WIN = 3

            def issue_pair(i):
                k, ei, j = wseq[i]
                src = ex["we_g"] if k == "g" else ex["we_u"]
                t_ = pfw.tile([P, 4 * I], F16, tag="wp", bufs=WIN,
                              name=f"wp{i}")
                eng = nc.sync if i % 2 == 0 else nc.scalar
                eng.dma_start(
                    t_[:].rearrange("p (four i) -> p four i", four=4),
                    src[ei, j * 4 * P:(j + 1) * 4 * P, :].rearrange(
                        "(four a) i -> a four i", a=P))
                wring[i] = t_

            # ======== Phase A: norm + transpose + QKV + rope ========
            with tc.tile_pool(name="pab", bufs=1) as pab:
                qT = [pab.tile([P, T], F16, tag=f"qT{j}", name=f"qT{j}")
                      for j in range(2)]
                kT = pab.tile([P, T], F16)
                v_tm = pab.tile([P, TB * D], F16)
                wo_sb = [pab.tile([P, H], F16, tag=f"wo{j}", name=f"wo{j}")
                         for j in range(2)]
                diagm = pab.tile([P, P], F16)

                with tc.tile_pool(name="pa", bufs=1) as pa, \
                     tc.tile_pool(name="pa2", bufs=2) as pa2:
                    hidbs = []
                    for b in range(TB):
                        t_ = pa2.tile([P, H], F16, tag="hidb", bufs=8,
                                      name=f"hidb{b}")
                        nc.sync.dma_start(t_[:], ex["hid"][b * P:(b + 1) * P, :])
                        hidbs.append(t_)
                    nc.sync.dma_start(wo_sb[0][:], ex["wo0"][:])
                    nc.sync.dma_start(wo_sb[1][:], ex["wo1"][:])
                    nc.sync.dma_start(diagm[:], ex["diagmask"][:])
                    cosq = pa.tile([D, T], F16)
                    sinq = pa.tile([D, T], F16)
                    cosk = pa.tile([D, T], F16)
                    sink = pa.tile([D, T], F16)
                    for t_, s_ in [(cosq, "cosq"), (sinq, "sinq"),
                                   (cosk, "cosk"), (sink, "sink")]:
                        nc.gpsimd.dma_start(t_[:], ex[s_][:])
                    permh = pa.tile([P, P], F16)
                    nc.gpsimd.dma_start(permh[:], ex["permh"][:])
                    wqkv_sb = pa.tile([P, HC * 512], F16)
                    nc.sync.dma_start(wqkv_sb[:], ex["w_qkv_pk"][:])
                    # persistent-weight prefetch (runs during attention)
                    nc.gpsimd.dma_start(wsg_sb[:], ex["ws_g_pk"][:])
                    nc.gpsimd.dma_start(wsu_sb[:], ex["ws_u_pk"][:])
                    for sp in range(2):
                        nc.gpsimd.dma_start(wsd_sb[sp][:],
                                            ex["ws_d"][sp * P:(sp + 1) * P, :])
                    nc.gpsimd.dma_start(gwr[:], ex["gate_w_pk"][:])
                    nc.gpsimd.dma_start(gate_b_sb[:], ex["gate_b"][:])
                    nc.gpsimd.dma_start(ut_sb[:], ex["ut_in"][:])
                    nc.gpsimd.dma_start(bc127[:], ex["bcast127"][:])
                    nc.gpsimd.dma_start(slb_sb[:], ex["slb_in"][:])
                    nc.gpsimd.dma_start(em[0][:], ex["emask0"][:])
                    nc.gpsimd.dma_start(em[1][:], ex["emask1"][:])
                    for i in range(2):
                        issue_pair(i)

                    x0T = [pa.tile([P, T], F16, tag=f"x0T{hc}",
                                   name=f"x0T{hc}") for hc in range(HC)]
                    qraw = [pa.tile([P, T], F16, tag=f"qraw{j}",
                                    name=f"qraw{j}") for j in range(2)]
                    kraw = pa.tile([P, T], F16)
                    vraw = pa.tile([P, T], F16)
                    dump = pa.tile([P, H], F32)

                    with tc.tile_pool(name="psA1", bufs=2, space="PSUM") as psA1, \
                         tc.tile_pool(name="psA2", bufs=1, space="PSUM") as psA2:
                        for n in range(2):
                            x0hs = []
                            for bb in range(TB // 2):
                                b = n * (TB // 2) + bb
                                hidb = hidbs[b]
                                ssum = pa2.tile([P, 1], F32, tag="ssum")
                                nc.scalar.activation(dump[:], hidb[:],
                                                     AF.Square,
                                                     accum_out=ssum[:, :1])
                                rms = pa2.tile([P, 1], F32, tag="rms")
                                nc.scalar.activation(rms[:], ssum[:], AF.Sqrt,
                                                     bias=eps_t[:, :1],
                                                     scale=1.0 / H)
                                inv = pa2.tile([P, 1], F32, tag="inv")
                                nc.vector.reciprocal(inv[:], rms[:])
                                x0h = pa2.tile([P, H], F16, tag="x0h", bufs=4,
                                               name=f"x0h{b}")
                                nc.vector.tensor_scalar_mul(x0h[:], hidb[:],
                                                            inv[:, :1])
                                x0hs.append(x0h)
                            sl = slice(n * 512, (n + 1) * 512)
                            for hc in range(HC):
                                tp = psA1.tile([P, 512], F16, tag="tpA")
                                for bb in range(4):
                                    nc.tensor.transpose(
                                        tp[:, bb * P:(bb + 1) * P],
                                        x0hs[bb][:, hc * P:(hc + 1) * P],
                                        identh[:])
                                if hc % 2 == 0:
                                    nc.vector.tensor_copy(x0T[hc][:, sl], tp[:])
                                else:
                                    nc.scalar.activation(x0T[hc][:, sl], tp[:],
                                                         AF.Copy)
                            ps4 = [psA2.tile([P, 512], F32, tag=f"qkv{j}",
                                             name=f"qkv{j}_{n}")
                                   for j in range(4)]
                            for hc in range(HC):
                                for j, c0 in enumerate([0, 128, 256, 384]):
                                    nc.tensor.matmul(
                                        ps4[j][:],
                                        wqkv_sb[:, hc * 512 + c0:
                                                hc * 512 + c0 + P],
                                        x0T[hc][:, sl],
                                        start=(hc == 0), stop=(hc == HC - 1))
                            for j, dst in enumerate([qraw[0], qraw[1],
                                                     kraw, vraw]):
                                if j % 2 == 0:
                                    nc.vector.tensor_copy(dst[:, sl], ps4[j][:])
                                else:
                                    nc.scalar.activation(dst[:, sl], ps4[j][:],
                                                         AF.Copy)

                    with tc.tile_pool(name="psA3", bufs=2, space="PSUM") as psA3, \
                         tc.tile_pool(name="psA4", bufs=2, space="PSUM") as psA4:
                        for src, dst, c_, s_ in [(qraw[0], qT[0], cosq, sinq),
                                                 (qraw[1], qT[1], cosq, sinq),
                                                 (kraw, kT, cosk, sink)]:
                            sw = psA3.tile([P, T], F32, tag="sw")
                            for nn in range(2):
                                sl = slice(nn * 512, (nn + 1) * 512)
                                nc.tensor.matmul(sw[:, sl], permh[:], src[:, sl],
                                                 start=True, stop=True)
                            t1 = pa2.tile([P, T], F16, tag="ropet1")
                            nc.gpsimd.tensor_mul(t1[:], src[:], c_[:])
                            t2 = pa2.tile([P, T], F16, tag="ropet2")
                            nc.vector.tensor_mul(t2[:], sw[:], s_[:])
                            nc.gpsimd.tensor_add(dst[:], t1[:], t2[:])
                        for g4 in range(2):
                            tp = psA4.tile([P, 512], F16, tag="tpV")
                            for bb in range(4):
                                b = g4 * 4 + bb
                                nc.tensor.transpose(
                                    tp[:, bb * P:(bb + 1) * P],
                                    vraw[:, b * P:(b + 1) * P], identh[:])
                            nc.vector.tensor_copy(
                                v_tm[:, g4 * 512:(g4 + 1) * 512], tp[:])

                # ==== Phase B: per-block attention + o_proj + chunked RS1 ==
                with tc.tile_pool(name="pb", bufs=1) as pb, \
                     tc.tile_pool(name="pb2", bufs=2) as pb2:
                    atn = [pb.tile([P, TB * P], F16, tag=f"atn{h}",
                                   name=f"atn{h}") for h in range(2)]
                    with tc.tile_pool(name="psBs", bufs=2, space="PSUM") as psBs, \
                         tc.tile_pool(name="psBt", bufs=1, space="PSUM") as psBt, \
                         tc.tile_pool(name="psAv", bufs=1, space="PSUM") as psAv, \
                         tc.tile_pool(name="psBp", bufs=1, space="PSUM") as psBp:
                        for qc in range(TB):
                            W = (qc + 1) * P
                            probs_h = []
                            for h in range(2):
                                sc = psBs.tile([P, T], F32, tag="sc")
                                for c0 in range(0, W, 512):
                                    c1 = min(c0 + 512, W)
                                    nc.tensor.matmul(
                                        sc[:, c0:c1],
                                        qT[h][:, qc * P:(qc + 1) * P],
                                        kT[:, c0:c1], start=True, stop=True)
                                nc.vector.tensor_tensor(
                                    out=sc[:, W - P:W], in0=sc[:, W - P:W],
                                    in1=diagm[:], op=ALU.add)
                                probs = pb2.tile([P, T], F16, tag="probs",
                                                 bufs=4)
                                ssum = pb2.tile([P, 1], F32, tag="esum")
                                nc.scalar.activation(probs[:, :W], sc[:, :W],
                                                     AF.Exp,
                                                     accum_out=ssum[:, :1])
                                rec = pb2.tile([P, 1], F32, tag="rec")
                                nc.vector.reciprocal(rec[:], ssum[:])
                                nc.vector.tensor_scalar_mul(probs[:, :W],
                                                            probs[:, :W],
                                                            rec[:, :1])
                                probs_h.append(probs)
                            oTb = []
                            av = psAv.tile([P, 2 * P], F32, tag="av",
                                           name=f"av_{qc}")
                            for h in range(2):
                                probs = probs_h[h]
                                for g4 in range(0, qc + 1, 4):
                                    cnt = min(4, qc + 1 - g4)
                                    tp = psBt.tile([P, 512], F16, tag="tpB")
                                    for i in range(cnt):
                                        kc = g4 + i
                                        nc.tensor.transpose(
                                            tp[:, i * P:(i + 1) * P],
                                            probs[:, kc * P:(kc + 1) * P],
                                            identh[:])
                                    dst = atn[h][:, g4 * P:(g4 + cnt) * P]
                                    if (qc + h) % 2 == 0:
                                        nc.vector.tensor_copy(dst,
                                                              tp[:, :cnt * P])
                                    else:
                                        nc.scalar.activation(dst,
                                                             tp[:, :cnt * P],
                                                             AF.Copy)
                                for kc in range(qc + 1):
                                    nc.tensor.matmul(
                                        av[:, h * P:(h + 1) * P],
                                        v_tm[:, kc * P:(kc + 1) * P],
                                        atn[h][:, kc * P:(kc + 1) * P],
                                        start=(kc == 0), stop=(kc == qc))
                                ot = pb2.tile([P, P], F16, tag=f"oTb{h}")
                                if h == 0:
                                    nc.vector.tensor_copy(
                                        ot[:], av[:, h * P:(h + 1) * P])
                                else:
                                    nc.scalar.activation(
                                        ot[:], av[:, h * P:(h + 1) * P],
                                        AF.Copy)
                                oTb.append(ot)
                            q_ = qc // 4
                            ro = (qc % 4) * P
                            ob = pb2.tile([P, H], F16, tag="ob")
                            for nn in range(2):
                                ps = psBp.tile([P, 1024], F32, tag="psO")
                                for h in range(2):
                                    for q2 in range(2):
                                        s2 = slice(q2 * 512, (q2 + 1) * 512)
                                        nc.tensor.matmul(
                                            ps[:, s2], oTb[h][:],
                                            wo_sb[h][:, nn * 1024 + q2 * 512:
                                                      nn * 1024 + (q2 + 1) * 512],
                                            start=(h == 0), stop=(h == 1))
                                dst = ob[:, nn * 1024:(nn + 1) * 1024]
                                if nn == 0:
                                    nc.vector.tensor_copy(dst, ps[:])
                                else:
                                    nc.scalar.activation(dst, ps[:], AF.Copy)
                            eng = nc.sync if qc % 2 == 0 else nc.scalar
                            eng.dma_start(rs1_in[q_][ro:ro + P, :], ob[:])
                            if qc % 4 == 3:
                                nc.gpsimd.collective_compute(
                                    "ReduceScatter", ALU.add,
                                    ins=[rs1_in[q_].opt()],
                                    outs=[rs1_out[q_].opt()],
                                    replica_groups=RG)

            # ======== Phase D: residual + norm + local router ========
            with tc.tile_pool(name="pd", bufs=1) as pd:
                hid_sl = pd.tile([P, H], F32)
                nc.sync.dma_start(hid_sl[:], ex["hid_slice"][:])
                attn_sl = pd.tile([P, H], F16)
                for q in range(NCH):
                    nc.sync.dma_start(attn_sl[q * SH:(q + 1) * SH, :],
                                      rs1_out[q][:])
                res_sb = pd.tile([P, H], F32)
                nc.vector.tensor_add(res_sb[:], hid_sl[:], attn_sl[:])
                nc.sync.dma_start(res_slice[:], res_sb[:])
                dump2 = pd.tile([P, H], F32)
                ssum = pd.tile([P, 1], F32)
                nc.scalar.activation(dump2[:], res_sb[:], AF.Square,
                                     accum_out=ssum[:, :1])
                rms = pd.tile([P, 1], F32)
                nc.scalar.activation(rms[:], ssum[:], AF.Sqrt,
                                     bias=eps_t[:, :1], scale=1.0 / H)
                inv = pd.tile([P, 1], F32)
                nc.vector.reciprocal(inv[:], rms[:])
                x_sl = pd.tile([P, H], F32)
                nc.vector.tensor_scalar_mul(x_sl[:], res_sb[:], inv[:, :1])
                payx = pd.tile([P, H], F16)
                nc.vector.tensor_copy(payx[:], x_sl[:])
                nc.scalar.dma_start(agx1_in[:, 0:H // 2], payx[:, H // 2:H])
                nc.sync.dma_start(agx2_in[:], payx[:, 0:H // 2])
                # local router on fp32 x
                xsT = pd.tile([P, HC * P], F32)
                with tc.tile_pool(name="psDt", bufs=2, space="PSUM") as psDt:
                    for g4 in range(4):
                        tp = psDt.tile([P, 512], F32, tag="tpD")
                        for i in range(4):
                            hc = g4 * 4 + i
                            nc.tensor.transpose(
                                tp[:, i * P:(i + 1) * P],
                                x_sl[:, hc * P:(hc + 1) * P], identf[:])
                        nc.vector.tensor_copy(
                            xsT[:, g4 * 512:(g4 + 1) * 512], tp[:])
                with tc.tile_pool(name="psDr", bufs=1, space="PSUM") as psDr:
                    lg = psDr.tile([P, E], F32, tag="lg")
                    for hc in range(HC):
                        nc.tensor.matmul(lg[:], xsT[:, hc * P:(hc + 1) * P],
                                         gwr[:, hc * E:(hc + 1) * E],
                                         start=(hc == 0), stop=(hc == HC - 1))
                    sig = pd.tile([P, E], F32)
                    nc.scalar.activation(sig[:], lg[:], AF.Sigmoid)
                sb_ = pd.tile([P, E], F32)
                nc.vector.tensor_add(sb_[:], sig[:], gate_b_sb[:])
                mx8 = pd.tile([P, 8], F32)
                nc.vector.max(out=mx8[:], in_=sb_[:])
                s1 = pd.tile([P, E], F32)
                nc.vector.tensor_tensor(out=s1[:], in0=sb_[:],
                                        in1=mx8[:, 0:1].to_broadcast([P, E]),
                                        op=ALU.is_equal)
                s2 = pd.tile([P, E], F32)
                nc.vector.tensor_tensor(out=s2[:], in0=sb_[:],
                                        in1=mx8[:, 1:2].to_broadcast([P, E]),
                                        op=ALU.is_equal)
                nc.vector.tensor_add(s1[:], s1[:], s2[:])
                nc.vector.tensor_scalar_min(s1[:], s1[:], 1.0)
                wa = pd.tile([P, E], F32)
                nc.vector.tensor_mul(wa[:], s1[:], sig[:])
                nrm = pd.tile([P, 1], F32)
                nc.vector.reduce_sum(nrm[:], wa[:], axis=AX.X)
                rec = pd.tile([P, 1], F32)
                nc.vector.reciprocal(rec[:], nrm[:])
                paw = pd.tile([P, WP], F16)
                nc.vector.tensor_scalar_mul(paw[:, 0:E], wa[:], rec[:, :1])
                selh = pd.tile([P, E], F16)
                nc.vector.tensor_copy(selh[:], s1[:])
                uml = pd.tile([P, E], F32)
                nc.vector.tensor_scalar(out=uml[:], in0=selh[:],
                                        scalar1=-BIG2, scalar2=BIG2,
                                        op0=ALU.mult, op1=ALU.add)
                pre_l = pd.tile([P, E], F16)
                with tc.tile_pool(name="psDp", bufs=1, space="PSUM") as psDp:
                    prp = psDp.tile([P, E], F32, tag="prp")
                    nc.tensor.matmul(prp[:], ut_sb[:], selh[:],
                                     start=True, stop=True)
                    nc.vector.tensor_copy(pre_l[:], prp[:])
                    nc.vector.tensor_tensor(out=paw[:, E:2 * E], in0=prp[:],
                                            in1=uml[:], op=ALU.add)
                    tbp = psDp.tile([P, E], F32, tag="tbp")
                    nc.tensor.matmul(tbp[:], bc127[:], pre_l[:],
                                     start=True, stop=True)
                    nc.vector.tensor_copy(paw[:, 2 * E:3 * E], tbp[:])
                nc.scalar.dma_start(agx1_in[:, H // 2:H // 2 + WP], paw[:])

            nc.gpsimd.collective_compute(
                "AllGather", ALU.bypass, ins=[agx1_in.opt()],
                outs=[x_tmA.opt()], replica_groups=RG)
            nc.gpsimd.collective_compute(
                "AllGather", ALU.bypass, ins=[agx2_in.opt()],
                outs=[x_tmB.opt()], replica_groups=RG)

            # ======== Phase X: token lists + x^T + shared + experts ======
            with tc.tile_pool(name="pg", bufs=1) as pg, \
                 tc.tile_pool(name="pg2", bufs=2) as pg2:
                xT = [pg.tile([P, T], F16, tag=f"xT{hc}", name=f"xT{hc}")
                      for hc in range(HC)]
                totals = pg.tile([8, E], F16)
                grank = [pg.tile([P, E], F32, tag=f"grank{b}",
                                 name=f"grank{b}") for b in range(TB)]
                wb0 = H // 2
                with tc.tile_pool(name="psXr", bufs=2, space="PSUM") as psXr:
                    nc.sync.dma_start(
                        totals[:].rearrange("b (o e) -> b o e", o=1),
                        x_tmA[:].rearrange("(b p) e -> b p e", p=P)[
                            :, 0:1, wb0 + 2 * E:wb0 + 3 * E])
                    wrbs = []
                    for b in range(TB):
                        wrb = pg2.tile([P, WP], F16, tag="wrb", bufs=8,
                                       name=f"wrb{b}")
                        nc.sync.dma_start(
                            wrb[:], x_tmA[b * P:(b + 1) * P, wb0:wb0 + WP])
                        wrbs.append(wrb)
                    for b in range(TB):
                        ofs = psXr.tile([P, E], F32, tag="ofs")
                        nc.tensor.matmul(ofs[:], slb_sb[:, b * P:(b + 1) * P],
                                         totals[:], start=True, stop=True)
                        nc.vector.tensor_tensor(out=grank[b][:],
                                                in0=wrbs[b][:, E:2 * E],
                                                in1=ofs[:], op=ALU.add)
                        gm = pg2.tile([P, E], F32, tag="gm")
                        nc.vector.tensor_scalar(out=gm[:], in0=grank[b][:],
                                                scalar1=float(CAP),
                                                scalar2=BIG,
                                                op0=ALU.is_gt, op1=ALU.mult)
                        nc.vector.tensor_add(grank[b][:], grank[b][:], gm[:])
                    sent = pg.tile([P, 1], I32)
                    nc.vector.memset(sent[:], 1000000)
                    for kk in range(2 * SL // P):
                        nc.sync.dma_start(tok_lists[kk * P:(kk + 1) * P, :],
                                          sent[:])
                    for b in range(TB):
                        tok = pg2.tile([P, 1], I32, tag="tok")
                        nc.gpsimd.iota(tok[:], pattern=[[0, 1]], base=b * P,
                                       channel_multiplier=1)
                        for ei in range(2):
                            gsel = pg2.tile([P, E], F32, tag="gsel")
                            nc.vector.tensor_mul(gsel[:], grank[b][:],
                                                 em[ei][:])
                            ridx = pg2.tile([P, 1], F32, tag="ridx")
                            nc.vector.reduce_sum(ridx[:], gsel[:], axis=AX.X)
                            nc.vector.tensor_scalar_add(ridx[:], ridx[:],
                                                        float(ei * SL - 1))
                            ridx_i = pg2.tile([P, 1], I32, tag="ridxi")
                            nc.vector.tensor_copy(ridx_i[:], ridx[:])
                            nc.gpsimd.indirect_dma_start(
                                out=tok_lists[:],
                                out_offset=bass.IndirectOffsetOnAxis(
                                    ap=ridx_i[:, :1], axis=0),
                                in_=tok[:], in_offset=None,
                                bounds_check=2 * SL - 1, oob_is_err=False)

                # x^T build: half A (hc 0-7) then half B (hc 8-15),
                # shared-expert gate pass interleaved
                hsh = [pg.tile([P, T], F16, tag=f"hs{sp}", name=f"hs{sp}")
                       for sp in range(2)]
                gsh = [pg.tile([P, T], F16, tag=f"gsh{sp}", name=f"gsh{sp}")
                       for sp in range(2)]
                with tc.tile_pool(name="psXt", bufs=2, space="PSUM") as psXt, \
                     tc.tile_pool(name="psS", bufs=1, space="PSUM") as psS:
                    gps = [psS.tile([P, T], F32, tag=f"sgp{sp}",
                                    name=f"sgp{sp}") for sp in range(2)]
                    for half, src_tm, hclo in [(0, x_tmB, 0), (1, x_tmA, 8)]:
                        for n in range(2):
                            xbs = []
                            for bb in range(4):
                                b = n * 4 + bb
                                xb = pg2.tile([P, H // 2], F16, tag="xb",
                                              bufs=4, name=f"xb{half}_{b}")
                                eng = nc.sync if bb % 2 == 0 else nc.scalar
                                eng.dma_start(
                                    xb[:], src_tm[b * P:(b + 1) * P,
                                                  0:H // 2])
                                xbs.append(xb)
                            sl = slice(n * 512, (n + 1) * 512)
                            for hc8 in range(8):
                                hc = hclo + hc8
                                tp = psXt.tile([P, 512], F16, tag="tpX")
                                for bb in range(4):
                                    nc.tensor.transpose(
                                        tp[:, bb * P:(bb + 1) * P],
                                        xbs[bb][:, hc8 * P:(hc8 + 1) * P],
                                        identh[:])
                                if hc % 2 == 0:
                                    nc.vector.tensor_copy(xT[hc][:, sl], tp[:])
                                else:
                                    nc.scalar.activation(xT[hc][:, sl], tp[:],
                                                         AF.Copy)
                        # shared gate pass for this half's hc range
                        for hc8 in range(8):
                            hc = hclo + hc8
                            for sp in range(2):
                                c0 = hc * ISC + sp * P
                                for nn in range(2):
                                    sl = slice(nn * 512, (nn + 1) * 512)
                                    nc.tensor.matmul(gps[sp][:, sl],
                                                     wsg_sb[:, c0:c0 + P],
                                                     xT[hc][:, sl],
                                                     start=(hc == 0),
                                                     stop=(hc == HC - 1))
                    for sp in range(2):
                        nc.vector.tensor_copy(gsh[sp][:], gps[sp][:])
                        nc.scalar.activation(gsh[sp][:], gsh[sp][:], AF.Silu)
                    ups = [psS.tile([P, T], F32, tag=f"sgp{sp}",
                                    name=f"sup{sp}") for sp in range(2)]
                    for hc in range(HC):
                        for sp in range(2):
                            c0 = hc * ISC + sp * P
                            for nn in range(2):
                                sl = slice(nn * 512, (nn + 1) * 512)
                                nc.tensor.matmul(ups[sp][:, sl],
                                                 wsu_sb[:, c0:c0 + P],
                                                 xT[hc][:, sl],
                                                 start=(hc == 0),
                                                 stop=(hc == HC - 1))
                    for sp in range(2):
                        nc.vector.tensor_mul(hsh[sp][:], gsh[sp][:],
                                             ups[sp][:])
                with tc.tile_pool(name="psS3", bufs=2, space="PSUM") as psS3:
                    for nn in range(2):
                        for tb2 in range(TB // 2):
                            sd = pg2.tile([P, 2048], F16, tag="sd", bufs=1)
                            for two in range(2):
                                tb_ = tb2 * 2 + two
                                ps3 = psS3.tile([P, 1024], F32, tag="psSd")
                                for sp in range(2):
                                    for q2 in range(2):
                                        s2 = slice(q2 * 512, (q2 + 1) * 512)
                                        nc.tensor.matmul(
                                            ps3[:, s2],
                                            hsh[sp][:, tb_ * P:(tb_ + 1) * P],
                                            wsd_sb[sp][:, nn * 1024 + q2 * 512:
                                                        nn * 1024 + (q2 + 1) * 512],
                                            start=(sp == 0), stop=(sp == 1))
                                dst = sd[:, two * 1024:(two + 1) * 1024]
                                if two == 0:
                                    nc.vector.tensor_copy(dst, ps3[:])
                                else:
                                    nc.scalar.activation(dst, ps3[:], AF.Copy)
                            eng = nc.sync if tb2 % 2 == 0 else nc.scalar
                            eng.dma_start(
                                rs2_in[nn][tb2 * 2 * P:(tb2 + 1) * 2 * P, :]
                                .rearrange("(two p) c -> p two c", two=2),
                                sd[:].rearrange("p (two c) -> p two c", two=2))

                # ======== Expert gathers + wcol (gpsimd ahead of PE) ======
                KL = [P, CAP - P]
                idx_sb = [[pg.tile([P if k == 0 else CAP - P, 1], I32,
                                   tag=f"idx{ei}_{k}",
                                   name=f"idx{ei}_{k}") for k in range(2)]
                          for ei in range(2)]
                gxT = [pg.tile([P, HC * SL], F16, tag=f"gxT{ei}",
                               name=f"gxT{ei}") for ei in range(2)]
                wcol = [[pg.tile([P if k == 0 else CAP - P, 1], F32,
                                 tag=f"wcol{ei}_{k}",
                                 name=f"wcol{ei}_{k}") for k in range(2)]
                        for ei in range(2)]
                with tc.tile_pool(name="psFt", bufs=2, space="PSUM") as psFt:
                    for ei in range(2):
                        gxA = [None, None]
                        gxB = [None, None]
                        for k in range(2):
                            nc.sync.dma_start(
                                idx_sb[ei][k][:],
                                tok_lists[ei * SL + k * P:
                                          ei * SL + k * P + KL[k], :])
                            ga_ = pg2.tile([P, H // 2 + WP], F16, tag="gxA",
                                           name=f"gxA{ei}_{k}")
                            nc.vector.memset(ga_[:KL[k], :], 0.0)
                            nc.gpsimd.indirect_dma_start(
                                out=ga_[:KL[k], :], out_offset=None,
                                in_=x_tmA[:],
                                in_offset=bass.IndirectOffsetOnAxis(
                                    ap=idx_sb[ei][k][:, :1], axis=0),
                                bounds_check=T - 1, oob_is_err=False)
                            gb_ = pg2.tile([P, H // 2], F16, tag="gxB",
                                           name=f"gxB{ei}_{k}")
                            nc.vector.memset(gb_[:KL[k], :], 0.0)
                            nc.gpsimd.indirect_dma_start(
                                out=gb_[:KL[k], :], out_offset=None,
                                in_=x_tmB[:],
                                in_offset=bass.IndirectOffsetOnAxis(
                                    ap=idx_sb[ei][k][:, :1], axis=0),
                                bounds_check=T - 1, oob_is_err=False)
                            gxA[k] = ga_
                            gxB[k] = gb_
                        # wcol first (vector-only, unblocks nothing behind)
                        for k in range(2):
                            wtmp_f = pg2.tile([P, E], F32, tag="wtmp")
                            wtmp = wtmp_f[:KL[k], :]
                            nc.vector.tensor_mul(
                                wtmp, gxA[k][:KL[k], wb0:wb0 + E],
                                em[ei][:KL[k], :])
                            nc.vector.reduce_sum(wcol[ei][k][:], wtmp,
                                                 axis=AX.X)
                        for hp in range(HC // 2):
                            tp = psFt.tile([P, 2 * CAP], F16, tag="tpF")
                            for i in range(2):
                                hc = hp * 2 + i
                                o0 = i * CAP
                                gsrc = gxB if hc < 8 else gxA
                                c0 = (hc % 8) * P
                                nc.tensor.transpose(
                                    tp[:, o0:o0 + P],
                                    gsrc[0][:, c0:c0 + P],
                                    identh[:])
                                nc.tensor.transpose(
                                    tp[:, o0 + P:o0 + CAP],
                                    gsrc[1][:KL[1], c0:c0 + P],
                                    identh[:KL[1], :KL[1]])
                            dst = gxT[ei][:].rearrange(
                                "p (hc c) -> p hc c", hc=HC)[
                                :, hp * 2:hp * 2 + 2, 0:CAP]
                            src = tp[:].rearrange("p (hc c) -> p hc c", hc=2)
                            if hp % 2 == 0:
                                nc.vector.tensor_copy(dst, src)
                            else:
                                nc.scalar.activation(dst, src, AF.Copy)

                # ======== Experts: I-partitioned gate/up ======
                h_sb = [pg.tile([P, IP * CAP], F16, tag=f"h_sb{ei}",
                                name=f"h_sb{ei}") for ei in range(2)]
                sg_sb = pg.tile([P, IP * CAP], F16)
                wd_res = [pg.tile([P, H], F16, tag=f"wd{e}_{ip}",
                                  name=f"wd{e}_{ip}")
                          for e in range(2) for ip in range(IP)]
                for ip in range(IP):
                    eng = nc.sync if ip % 2 == 0 else nc.scalar
                    eng.dma_start(wd_res[ip][:],
                                  ex["we_d"][0, ip * P:(ip + 1) * P, :])
                wi = 2
                for ei in range(2):
                    if ei == 1:
                        for ip in range(IP):
                            eng = nc.sync if ip % 2 == 0 else nc.scalar
                            eng.dma_start(
                                wd_res[IP + ip][:],
                                ex["we_d"][1, ip * P:(ip + 1) * P, :])
                    with tc.tile_pool(name=f"psF1{ei}", bufs=1,
                                      space="PSUM") as psF1:
                        acc = [psF1.tile([P, 256], F32, tag=f"acc{ip}",
                                         name=f"acc{ip}_{ei}")
                               for ip in range(IP)]
                        for kind in ("g", "u"):
                            base = (0 if kind == "g" else NJ) + ei * 2 * NJ
                            for j in range(NJ):
                                wp = wring[base + j]
                                if wi < len(wseq):
                                    issue_pair(wi)
                                    wi += 1
                                for four in range(4):
                                    hc = 4 * j + four
                                    for ip in range(IP):
                                        nc.tensor.matmul(
                                            acc[ip][:, :CAP],
                                            wp[:, four * I + ip * P:
                                               four * I + (ip + 1) * P],
                                            gxT[ei][:, hc * SL:hc * SL + CAP],
                                            start=(hc == 0),
                                            stop=(hc == HC - 1))
                            if kind == "g":
                                for ip in range(IP):
                                    nc.scalar.activation(
                                        sg_sb[:, ip * CAP:(ip + 1) * CAP],
                                        acc[ip][:, :CAP], AF.Silu)
                            else:
                                for ip in range(IP):
                                    nc.vector.tensor_mul(
                                        h_sb[ei][:, ip * CAP:(ip + 1) * CAP],
                                        sg_sb[:, ip * CAP:(ip + 1) * CAP],
                                        acc[ip][:, :CAP])

                # ======== Down-projections, column-half outer + RS2 ======
                with tc.tile_pool(name="psF3", bufs=2, space="PSUM") as psF3:
                    for nn in range(2):
                        for ei in range(2):
                            for k in range(2):
                                kl = KL[k]
                                koff = k * P
                                psd = psF3.tile([P, 1024], F32, tag="fd")
                                for ip in range(IP):
                                    c0 = ip * CAP + koff
                                    for q2 in range(2):
                                        s2 = slice(q2 * 512, (q2 + 1) * 512)
                                        nc.tensor.matmul(
                                            psd[:kl, s2],
                                            h_sb[ei][:, c0:c0 + kl],
                                            wd_res[ei * IP + ip][
                                                :, nn * 1024 + q2 * 512:
                                                nn * 1024 + (q2 + 1) * 512],
                                            start=(ip == 0),
                                            stop=(ip == IP - 1))
                                out_f = pg2.tile([P, 1024], F16, tag="outsb",
                                                 name=f"outsb{nn}{ei}{k}")
                                nc.vector.tensor_scalar_mul(
                                    out_f[:kl, :], psd[:kl, :],
                                    wcol[ei][k][:, :1])
                                nc.gpsimd.indirect_dma_start(
                                    out=rs2_in[nn][:],
                                    out_offset=bass.IndirectOffsetOnAxis(
                                        ap=idx_sb[ei][k][:, :1], axis=0),
                                    in_=out_f[:kl, :], in_offset=None,
                                    bounds_check=T - 1, oob_is_err=False,
                                    compute_op=ALU.add)
                        nc.gpsimd.collective_compute(
                            "ReduceScatter", ALU.add, ins=[rs2_in[nn].opt()],
                            outs=[rs2_out[nn].opt()], replica_groups=RG)

            for nn in range(2):
                nc.sync.dma_start(out_slice[:, nn * 1024:(nn + 1) * 1024],
                                  rs2_out[nn][:])


_CACHE = {}


def _build():
    key = "nc"
    if key in _CACHE:
        return _CACHE[key]
    nc = bacc.Bacc("TRN2", target_bir_lowering=False, debug=False,
                   num_devices=NCN)
    with tile.TileContext(nc) as tc:
        _emit(nc, tc)
    nc.compile()
    _CACHE[key] = nc
    return nc


def _perm_rows(c):
    return np.concatenate([np.arange(q * CH + c * SH, q * CH + (c + 1) * SH)
                           for q in range(NCH)])


def _host_prep(inputs):
    f16 = np.float16
    pos = np.asarray(inputs["positions"]).astype(np.float64)
    hid = np.asarray(inputs["hidden_states"], np.float32)
    w_in = np.asarray(inputs["w_in_ln"], np.float32)
    w_post = np.asarray(inputs["w_post_ln"], np.float32)
    wq = np.asarray(inputs["wq"], np.float32) * w_in[:, None]
    wk = np.asarray(inputs["wk"], np.float32) * w_in[:, None]
    wv = np.asarray(inputs["wv"], np.float32) * w_in[:, None]
    wo = np.asarray(inputs["wo"], np.float32)
    gate_w = np.asarray(inputs["gate_w"], np.float32) * w_post[None, :]
    gate_b = np.asarray(inputs["gate_bias"], np.float32).reshape(1, E)
    we_g = (np.asarray(inputs["we_gate"], np.float32)
            * w_post[None, :, None]).astype(f16)
    we_u = (np.asarray(inputs["we_up"], np.float32)
            * w_post[None, :, None]).astype(f16)
    we_d = np.asarray(inputs["we_down"], np.float32).astype(f16)
    ws_g = np.asarray(inputs["ws_gate"], np.float32) * w_post[:, None]
    ws_u = np.asarray(inputs["ws_up"], np.float32) * w_post[:, None]
    ws_d = np.asarray(inputs["ws_down"], np.float32).astype(f16)

    inv_freq = 1.0 / (THETA ** (np.arange(0, D, 2, dtype=np.float64) / D))
    f = pos[None, :] * inv_freq[:, None]
    cos2, sin2 = np.cos(f), np.sin(f)
    cosT = np.repeat(cos2, 2, axis=0).astype(np.float32)
    sinT = np.empty((D, T), np.float32)
    sinT[0::2] = -sin2
    sinT[1::2] = sin2
    s = 1.0 / np.sqrt(D)
    cosq, sinq = (cosT * s).astype(f16), (sinT * s).astype(f16)
    cosk, sink = cosT.astype(f16), sinT.astype(f16)

    ii = np.arange(P)
    diagmask = np.where(ii[:, None] >= ii[None, :], 0.0, NEG).astype(f16)
    ident = np.eye(P, dtype=np.float32)
    ut_in = np.triu(np.ones((P, P), np.float32)).astype(f16)
    slb_in = np.zeros((8, TB * P), np.float32)
    for b in range(TB):
        slb_in[:b, b * P:(b + 1) * P] = 1.0
    slb_in = slb_in.astype(f16)
    bc127 = np.zeros((P, P), np.float32)
    bc127[127, :] = 1.0
    bc127 = bc127.astype(f16)
    perm = np.zeros((P, P), np.float32)
    for i in range(0, P, 2):
        perm[i, i + 1] = 1.0
        perm[i + 1, i] = 1.0

    def pack_pk(w, width):  # w: [H, width]
        return np.ascontiguousarray(
            w.reshape(HC, P, width).transpose(1, 0, 2).reshape(P, HC * width))

    gate_w_pk = pack_pk(gate_w.T.astype(np.float32), E)

    maps = []
    for c in range(NCN):
        g = c // 2
        w_qkv = pack_pk(np.concatenate([
            wq[:, 2 * c * D:(2 * c + 1) * D],
            wq[:, (2 * c + 1) * D:(2 * c + 2) * D],
            wk[:, g * D:(g + 1) * D],
            wv[:, g * D:(g + 1) * D]], axis=1), 512).astype(f16)
        em0 = np.zeros((P, E), np.float32)
        em0[:, 2 * c] = 1.0
        em1 = np.zeros((P, E), np.float32)
        em1[:, 2 * c + 1] = 1.0
        maps.append({
            "hid": hid.astype(f16),
            "hid_slice": np.ascontiguousarray(hid[_perm_rows(c)]),
            "w_qkv_pk": w_qkv,
            "wo0": np.ascontiguousarray(wo[2 * c * D:(2 * c + 1) * D]).astype(f16),
            "wo1": np.ascontiguousarray(
                wo[(2 * c + 1) * D:(2 * c + 2) * D]).astype(f16),
            "cosq": cosq, "sinq": sinq, "cosk": cosk, "sink": sink,
            "permh": perm.astype(f16), "identh_in": ident.astype(f16),
            "identr_in": ident, "diagmask": diagmask,
            "gate_w_pk": gate_w_pk,
            "gate_b": np.broadcast_to(gate_b, (P, E)).astype(np.float32).copy(),
            "emask0": em0, "emask1": em1,
            "ut_in": ut_in, "slb_in": slb_in, "bcast127": bc127,
            "ws_g_pk": pack_pk(
                ws_g[:, c * ISC:(c + 1) * ISC].astype(np.float32), ISC
            ).astype(f16),
            "ws_u_pk": pack_pk(
                ws_u[:, c * ISC:(c + 1) * ISC].astype(np.float32), ISC
            ).astype(f16),
            "ws_d": np.ascontiguousarray(ws_d[c * ISC:(c + 1) * ISC]),
            "we_g": np.ascontiguousarray(we_g[2 * c:2 * c + 2]),
            "we_u": np.ascontiguousarray(we_u[2 * c:2 * c + 2]),
            "we_d": np.ascontiguousarray(we_d[2 * c:2 * c + 2]),
        })
    return maps


def kernel(trace=False, **inputs):
    nc = _build()
    maps = _host_prep(inputs)
    res = bass_utils.run_bass_kernel_spmd(
        nc, maps, core_ids=list(range(NCN)), trace=trace)
    out = np.empty((T, H), np.float32)
    resid = np.empty((T, H), np.float32)
    for c in range(NCN):
        rows = _perm_rows(c)
        out[rows] = res.results[c]["out_slice"].astype(np.float32)
        resid[rows] = res.results[c]["res_slice"]
    kernel.last_results = res
    return out, resid


# revision 29
# speedup vs baseline: 1.0688x; 1.0526x over previous
"""Ernie4 decoder layer (RMSNorm + GQA attention + shared expert + 16-expert
top-2 MoE) on 8 Trainium2 NeuronCores.

v7 (pipelined collectives):
  - Attention head-parallel, processed query-block-major: per 128-token
    block both heads' scores/softmax/AV and the o_proj run immediately,
    feeding 2 token-chunked ReduceScatters that fire DURING attention.
    Token ownership becomes permuted (32-row shards per chunk); the host
    permutes hid_slice in and unpermutes outputs.
  - x AllGather splits in two column chunks; collectives execute in
    readiness order, so the early chunk carries the x lo-half (ready at
    the norm) and the late chunk carries [x hi-half | router payload]
    (ready after the router). x^T build and the shared-expert gate pass
    consume the halves in arrival order.
  - Expert capacity 160 (max actual load ~155); gate/up I-partitioned
    (no h transposes); weight-scale (wcol) gathers run before expert
    compute so the gpsimd queue never blocks the down-projections.
  - MoE/shared/router weights prefetch on the gpsimd software-DGE queues
    during attention; expert weights stream as 1MB pairs alternating the
    two HWDGE queues; down-projections run column-half-outer feeding 2
    chunked ReduceScatters so RS2a overlaps the second half's compute.
"""
import sys
sys.path.insert(0, "/opt/trn_rl_repo")

import numpy as np

import concourse.bass as bass
import concourse.bacc as bacc
import concourse.tile as tile
import concourse.mybir as mybir
from concourse import bass_utils

dt = mybir.dt
F32 = dt.float32
F16 = dt.float16
I32 = dt.int32
AF = mybir.ActivationFunctionType
ALU = mybir.AluOpType
AX = mybir.AxisListType

T, H, NH, NKV, D = 1024, 2048, 16, 4, 128
E, I, IS = 16, 1024, 2048
ISC = IS // 8
EPS = 1e-6
THETA = 10000.0
NCN = 8
P = 128
TB = T // P
HC = H // P
IP = I // P
CAP = 160               # per-expert compute capacity (rank mask)
SL = 256                # per-expert list-slot spacing (square layouts)
NCH = 2                 # RS1 token chunks
CH = T // NCH           # 256 tokens per chunk
SH = CH // NCN          # 32-row per-core shard per chunk
WP = 3 * E              # router payload width
BIG = 1.0e6
BIG2 = 30000.0
NEG = -30000.0
RG = [list(range(NCN))]


def _emit(nc, tc):
    ex = {}
    for name, shape, d in [
        ("hid", [T, H], F16), ("hid_slice", [P, H], F32),
        ("w_qkv_pk", [P, HC * 512], F16),
        ("wo0", [D, H], F16), ("wo1", [D, H], F16),
        ("cosq", [D, T], F16), ("sinq", [D, T], F16),
        ("cosk", [D, T], F16), ("sink", [D, T], F16),
        ("permh", [P, P], F16), ("identh_in", [P, P], F16),
        ("identr_in", [P, P], F32), ("diagmask", [P, P], F16),
        ("gate_w_pk", [P, HC * E], F32), ("gate_b", [P, E], F32),
        ("emask0", [P, E], F32), ("emask1", [P, E], F32),
        ("ut_in", [P, P], F16), ("slb_in", [8, TB * P], F16),
        ("bcast127", [P, P], F16),
        ("ws_g_pk", [P, HC * ISC], F16), ("ws_u_pk", [P, HC * ISC], F16),
        ("ws_d", [ISC, H], F16),
        ("we_g", [2, H, I], F16), ("we_u", [2, H, I], F16),
        ("we_d", [2, I, H], F16),
    ]:
        ex[name] = nc.dram_tensor(name, shape, d, kind="ExternalInput").ap()
    out_slice = nc.dram_tensor("out_slice", [P, H], F16, kind="ExternalOutput").ap()
    res_slice = nc.dram_tensor("res_slice", [P, H], F32, kind="ExternalOutput").ap()

    with tc.tile_pool(name="pp", bufs=1) as pp, \
         tc.tile_pool(name="dram", bufs=1, space="DRAM") as dram:
        rs1_in = [dram.tile([CH, H], F16, tag=f"rs1i{q}", name=f"rs1i{q}")
                  for q in range(NCH)]
        rs1_out = [dram.tile([SH, H], F16, tag=f"rs1o{q}", name=f"rs1o{q}")
                   for q in range(NCH)]
        agx1_in = dram.tile([P, H // 2 + WP], F16)
        agx2_in = dram.tile([P, H // 2], F16)
        x_tmA = dram.tile([T, H // 2 + WP], F16, addr_space="Shared")
        x_tmB = dram.tile([T, H // 2], F16, addr_space="Shared")
        tok_lists = dram.tile([2 * SL, 1], I32)
        rs2_in = [dram.tile([T, H // 2], F16, tag=f"rs2i{nn}",
                            name=f"rs2i{nn}") for nn in range(2)]
        rs2_out = [dram.tile([P, H // 2], F16, tag=f"rs2o{nn}",
                             name=f"rs2o{nn}") for nn in range(2)]

        identh = pp.tile([P, P], F16)
        nc.sync.dma_start(identh[:], ex["identh_in"][:])
        identf = pp.tile([P, P], F32)
        nc.sync.dma_start(identf[:], ex["identr_in"][:])
        eps_t = pp.tile([P, 1], F32)
        nc.vector.memset(eps_t[:], EPS)

        # ======== persistent weight pool (prefetched during attention) ====
        with tc.tile_pool(name="pw", bufs=1) as pw, \
             tc.tile_pool(name="pfw", bufs=1) as pfw:
            wsg_sb = pw.tile([P, HC * ISC], F16)
            wsu_sb = pw.tile([P, HC * ISC], F16)
            wsd_sb = [pw.tile([P, H], F16, tag=f"wsd{sp}", name=f"wsd{sp}")
                      for sp in range(2)]
            gwr = pw.tile([P, HC * E], F32)
            gate_b_sb = pw.tile([P, E], F32)
            ut_sb = pw.tile([P, P], F16)
            bc127 = pw.tile([P, P], F16)
            slb_sb = pw.tile([8, TB * P], F16)
            em = [pw.tile([P, E], F32, tag=f"em{e}", name=f"em{e}")
                  for e in range(2)]

            NJ = IP // 2  # 4 hc rows per 1MB pair load
            wseq = [(k, ei, j) for ei in range(2) for k in ("g", "u")
                    for j in range(NJ)]
            wring = {}
            WIN = 3

            def issue_pair(i):
                k, ei, j = wseq[i]
                src = ex["we_g"] if k == "g" else ex["we_u"]
                t_ = pfw.tile([P, 4 * I], F16, tag="wp", bufs=WIN,
                              name=f"wp{i}")
                eng = nc.sync if i % 2 == 0 else nc.scalar
                eng.dma_start(
                    t_[:].rearrange("p (four i) -> p four i", four=4),
                    src[ei, j * 4 * P:(j + 1) * 4 * P, :].rearrange(
                        "(four a) i -> a four i", a=P))
                wring[i] = t_

            # ======== Phase A: norm + transpose + QKV + rope ========
            with tc.tile_pool(name="pab", bufs=1) as pab:
                qT = [pab.tile([P, T], F16, tag=f"qT{j}", name=f"qT{j}")
                      for j in range(2)]
                kT = pab.tile([P, T], F16)
                v_tm = pab.tile([P, TB * D], F16)
                wo_sb = [pab.tile([P, H], F16, tag=f"wo{j}", name=f"wo{j}")
                         for j in range(2)]
                diagm = pab.tile([P, P], F16)

                with tc.tile_pool(name="pa", bufs=1) as pa, \
                     tc.tile_pool(name="pa2", bufs=2) as pa2:
                    hidbs = []
                    for b in range(TB):
                        t_ = pa2.tile([P, H], F16, tag="hidb", bufs=8,
                                      name=f"hidb{b}")
                        nc.sync.dma_start(t_[:], ex["hid"][b * P:(b + 1) * P, :])
                        hidbs.append(t_)
                    nc.sync.dma_start(wo_sb[0][:], ex["wo0"][:])
                    nc.sync.dma_start(wo_sb[1][:], ex["wo1"][:])
                    nc.sync.dma_start(diagm[:], ex["diagmask"][:])
                    cosq = pa.tile([D, T], F16)
                    sinq = pa.tile([D, T], F16)
                    cosk = pa.tile([D, T], F16)
                    sink = pa.tile([D, T], F16)
                    for t_, s_ in [(cosq, "cosq"), (sinq, "sinq"),
                                   (cosk, "cosk"), (sink, "sink")]:
                        nc.gpsimd.dma_start(t_[:], ex[s_][:])
                    permh = pa.tile([P, P], F16)
                    nc.gpsimd.dma_start(permh[:], ex["permh"][:])
                    wqkv_sb = pa.tile([P, HC * 512], F16)
                    nc.sync.dma_start(wqkv_sb[:], ex["w_qkv_pk"][:])
                    # persistent-weight prefetch (runs during attention)
                    nc.gpsimd.dma_start(wsg_sb[:], ex["ws_g_pk"][:])
                    nc.gpsimd.dma_start(wsu_sb[:], ex["ws_u_pk"][:])
                    for sp in range(2):
                        nc.gpsimd.dma_start(wsd_sb[sp][:],
                                            ex["ws_d"][sp * P:(sp + 1) * P, :])
                    nc.gpsimd.dma_start(gwr[:], ex["gate_w_pk"][:])
                    nc.gpsimd.dma_start(gate_b_sb[:], ex["gate_b"][:])
                    nc.gpsimd.dma_start(ut_sb[:], ex["ut_in"][:])
                    nc.gpsimd.dma_start(bc127[:], ex["bcast127"][:])
                    nc.gpsimd.dma_start(slb_sb[:], ex["slb_in"][:])
                    nc.gpsimd.dma_start(em[0][:], ex["emask0"][:])
                    nc.gpsimd.dma_start(em[1][:], ex["emask1"][:])
                    for i in range(2):
                        issue_pair(i)

                    x0T = [pa.tile([P, T], F16, tag=f"x0T{hc}",
                                   name=f"x0T{hc}") for hc in range(HC)]
                    qraw = [pa.tile([P, T], F16, tag=f"qraw{j}",
                                    name=f"qraw{j}") for j in range(2)]
                    kraw = pa.tile([P, T], F16)
                    vraw = pa.tile([P, T], F16)
                    dump = pa.tile([P, H], F32)

                    with tc.tile_pool(name="psA1", bufs=2, space="PSUM") as psA1, \
                         tc.tile_pool(name="psA2", bufs=1, space="PSUM") as psA2:
                        for n in range(2):
                            x0hs = []
                            for bb in range(TB // 2):
                                b = n * (TB // 2) + bb
                                hidb = hidbs[b]
                                ssum = pa2.tile([P, 1], F32, tag="ssum")
                                nc.scalar.activation(dump[:], hidb[:],
                                                     AF.Square,
                                                     accum_out=ssum[:, :1])
                                rms = pa2.tile([P, 1], F32, tag="rms")
                                nc.scalar.activation(rms[:], ssum[:], AF.Sqrt,
                                                     bias=eps_t[:, :1],
                                                     scale=1.0 / H)
                                inv = pa2.tile([P, 1], F32, tag="inv")
                                nc.vector.reciprocal(inv[:], rms[:])
                                x0h = pa2.tile([P, H], F16, tag="x0h", bufs=4,
                                               name=f"x0h{b}")
                                nc.vector.tensor_scalar_mul(x0h[:], hidb[:],
                                                            inv[:, :1])
                                x0hs.append(x0h)
                            sl = slice(n * 512, (n + 1) * 512)
                            for hc in range(HC):
                                tp = psA1.tile([P, 512], F16, tag="tpA")
                                for bb in range(4):
                                    nc.tensor.transpose(
                                        tp[:, bb * P:(bb + 1) * P],
                                        x0hs[bb][:, hc * P:(hc + 1) * P],
                                        identh[:])
                                if hc % 2 == 0:
                                    nc.vector.tensor_copy(x0T[hc][:, sl], tp[:])
                                else:
                                    nc.scalar.activation(x0T[hc][:, sl], tp[:],
                                                         AF.Copy)
                            ps4 = [psA2.tile([P, 512], F32, tag=f"qkv{j}",
                                             name=f"qkv{j}_{n}")
                                   for j in range(4)]
                            for hc in range(HC):
                                for j, c0 in enumerate([0, 128, 256, 384]):
                                    nc.tensor.matmul(
                                        ps4[j][:],
                                        wqkv_sb[:, hc * 512 + c0:
                                                hc * 512 + c0 + P],
                                        x0T[hc][:, sl],
                                        start=(hc == 0), stop=(hc == HC - 1))
                            for j, dst in enumerate([qraw[0], qraw[1],
                                                     kraw, vraw]):
                                if j % 2 == 0:
                                    nc.vector.tensor_copy(dst[:, sl], ps4[j][:])
                                else:
                                    nc.scalar.activation(dst[:, sl], ps4[j][:],
                                                         AF.Copy)

                    with tc.tile_pool(name="psA3", bufs=2, space="PSUM") as psA3, \
                         tc.tile_pool(name="psA4", bufs=2, space="PSUM") as psA4:
                        for src, dst, c_, s_ in [(kraw, kT, cosk, sink),
                                                 (qraw[0], qT[0], cosq, sinq),
                                                 (qraw[1], qT[1], cosq, sinq)]:
                            sw = psA3.tile([P, T], F32, tag="sw")
                            for nn in range(2):
                                sl = slice(nn * 512, (nn + 1) * 512)
                                nc.tensor.matmul(sw[:, sl], permh[:], src[:, sl],
                                                 start=True, stop=True)
                            t1 = pa2.tile([P, T], F16, tag="ropet1")
                            nc.gpsimd.tensor_mul(t1[:], src[:], c_[:])
                            t2 = pa2.tile([P, T], F16, tag="ropet2")
                            nc.vector.tensor_mul(t2[:], sw[:], s_[:])
                            nc.gpsimd.tensor_add(dst[:], t1[:], t2[:])
                        for g4 in range(2):
                            tp = psA4.tile([P, 512], F16, tag="tpV")
                            for bb in range(4):
                                b = g4 * 4 + bb
                                nc.tensor.transpose(
                                    tp[:, bb * P:(bb + 1) * P],
                                    vraw[:, b * P:(b + 1) * P], identh[:])
                            nc.vector.tensor_copy(
                                v_tm[:, g4 * 512:(g4 + 1) * 512], tp[:])

                # ==== Phase B: per-block attention + o_proj + chunked RS1 ==
                with tc.tile_pool(name="pb", bufs=1) as pb, \
                     tc.tile_pool(name="pb2", bufs=2) as pb2:
                    atn = [pb.tile([P, TB * P], F16, tag=f"atn{h}",
                                   name=f"atn{h}") for h in range(2)]
                    with tc.tile_pool(name="psBs", bufs=2, space="PSUM") as psBs, \
                         tc.tile_pool(name="psBt", bufs=1, space="PSUM") as psBt, \
                         tc.tile_pool(name="psAv", bufs=1, space="PSUM") as psAv, \
                         tc.tile_pool(name="psBp", bufs=1, space="PSUM") as psBp:
                        for qc in range(TB):
                            W = (qc + 1) * P
                            probs_h = []
                            for h in range(2):
                                sc = psBs.tile([P, T], F32, tag="sc")
                                for c0 in range(0, W, 512):
                                    c1 = min(c0 + 512, W)
                                    nc.tensor.matmul(
                                        sc[:, c0:c1],
                                        qT[h][:, qc * P:(qc + 1) * P],
                                        kT[:, c0:c1], start=True, stop=True)
                                nc.vector.tensor_tensor(
                                    out=sc[:, W - P:W], in0=sc[:, W - P:W],
                                    in1=diagm[:], op=ALU.add)
                                probs = pb2.tile([P, T], F16, tag="probs",
                                                 bufs=4)
                                ssum = pb2.tile([P, 1], F32, tag="esum")
                                nc.scalar.activation(probs[:, :W], sc[:, :W],
                                                     AF.Exp,
                                                     accum_out=ssum[:, :1])
                                rec = pb2.tile([P, 1], F32, tag="rec")
                                nc.vector.reciprocal(rec[:], ssum[:])
                                nc.vector.tensor_scalar_mul(probs[:, :W],
                                                            probs[:, :W],
                                                            rec[:, :1])
                                probs_h.append(probs)
                            oTb = []
                            av = psAv.tile([P, 2 * P], F32, tag="av",
                                           name=f"av_{qc}")
                            for h in range(2):
                                probs = probs_h[h]
                                for g4 in range(0, qc + 1, 4):
                                    cnt = min(4, qc + 1 - g4)
                                    tp = psBt.tile([P, 512], F16, tag="tpB")
                                    for i in range(cnt):
                                        kc = g4 + i
                                        nc.tensor.transpose(
                                            tp[:, i * P:(i + 1) * P],
                                            probs[:, kc * P:(kc + 1) * P],
                                            identh[:])
                                    dst = atn[h][:, g4 * P:(g4 + cnt) * P]
                                    if (qc + h) % 2 == 0:
                                        nc.vector.tensor_copy(dst,
                                                              tp[:, :cnt * P])
                                    else:
                                        nc.scalar.activation(dst,
                                                             tp[:, :cnt * P],
                                                             AF.Copy)
                                for kc in range(qc + 1):
                                    nc.tensor.matmul(
                                        av[:, h * P:(h + 1) * P],
                                        v_tm[:, kc * P:(kc + 1) * P],
                                        atn[h][:, kc * P:(kc + 1) * P],
                                        start=(kc == 0), stop=(kc == qc))
                                ot = pb2.tile([P, P], F16, tag=f"oTb{h}")
                                if h == 0:
                                    nc.vector.tensor_copy(
                                        ot[:], av[:, h * P:(h + 1) * P])
                                else:
                                    nc.scalar.activation(
                                        ot[:], av[:, h * P:(h + 1) * P],
                                        AF.Copy)
                                oTb.append(ot)
                            q_ = qc // 4
                            ro = (qc % 4) * P
                            ob = pb2.tile([P, H], F16, tag="ob")
                            for nn in range(2):
                                ps = psBp.tile([P, 1024], F32, tag="psO")
                                for h in range(2):
                                    for q2 in range(2):
                                        s2 = slice(q2 * 512, (q2 + 1) * 512)
                                        nc.tensor.matmul(
                                            ps[:, s2], oTb[h][:],
                                            wo_sb[h][:, nn * 1024 + q2 * 512:
                                                      nn * 1024 + (q2 + 1) * 512],
                                            start=(h == 0), stop=(h == 1))
                                dst = ob[:, nn * 1024:(nn + 1) * 1024]
                                if nn == 0:
                                    nc.vector.tensor_copy(dst, ps[:])
                                else:
                                    nc.scalar.activation(dst, ps[:], AF.Copy)
                            eng = nc.sync if qc % 2 == 0 else nc.scalar
                            eng.dma_start(rs1_in[q_][ro:ro + P, :], ob[:])
                            if qc % 4 == 3:
                                nc.gpsimd.collective_compute(
                                    "ReduceScatter", ALU.add,
                                    ins=[rs1_in[q_].opt()],
                                    outs=[rs1_out[q_].opt()],
                                    replica_groups=RG)

            # ======== Phase D: residual + norm + local router ========
            with tc.tile_pool(name="pd", bufs=1) as pd:
                hid_sl = pd.tile([P, H], F32)
                nc.sync.dma_start(hid_sl[:], ex["hid_slice"][:])
                attn_sl = pd.tile([P, H], F16)
                for q in range(NCH):
                    nc.sync.dma_start(attn_sl[q * SH:(q + 1) * SH, :],
                                      rs1_out[q][:])
                res_sb = pd.tile([P, H], F32)
                nc.vector.tensor_add(res_sb[:], hid_sl[:], attn_sl[:])
                nc.sync.dma_start(res_slice[:], res_sb[:])
                dump2 = pd.tile([P, H], F32)
                ssum = pd.tile([P, 1], F32)
                nc.scalar.activation(dump2[:], res_sb[:], AF.Square,
                                     accum_out=ssum[:, :1])
                rms = pd.tile([P, 1], F32)
                nc.scalar.activation(rms[:], ssum[:], AF.Sqrt,
                                     bias=eps_t[:, :1], scale=1.0 / H)
                inv = pd.tile([P, 1], F32)
                nc.vector.reciprocal(inv[:], rms[:])
                x_sl = pd.tile([P, H], F32)
                nc.vector.tensor_scalar_mul(x_sl[:], res_sb[:], inv[:, :1])
                payx = pd.tile([P, H], F16)
                nc.vector.tensor_copy(payx[:], x_sl[:])
                nc.scalar.dma_start(agx1_in[:, 0:H // 2], payx[:, H // 2:H])
                nc.sync.dma_start(agx2_in[:], payx[:, 0:H // 2])
                # local router on fp32 x
                xsT = pd.tile([P, HC * P], F32)
                with tc.tile_pool(name="psDt", bufs=2, space="PSUM") as psDt:
                    for g4 in range(4):
                        tp = psDt.tile([P, 512], F32, tag="tpD")
                        for i in range(4):
                            hc = g4 * 4 + i
                            nc.tensor.transpose(
                                tp[:, i * P:(i + 1) * P],
                                x_sl[:, hc * P:(hc + 1) * P], identf[:])
                        nc.vector.tensor_copy(
                            xsT[:, g4 * 512:(g4 + 1) * 512], tp[:])
                with tc.tile_pool(name="psDr", bufs=1, space="PSUM") as psDr:
                    lg = psDr.tile([P, E], F32, tag="lg")
                    for hc in range(HC):
                        nc.tensor.matmul(lg[:], xsT[:, hc * P:(hc + 1) * P],
                                         gwr[:, hc * E:(hc + 1) * E],
                                         start=(hc == 0), stop=(hc == HC - 1))
                    sig = pd.tile([P, E], F32)
                    nc.scalar.activation(sig[:], lg[:], AF.Sigmoid)
                sb_ = pd.tile([P, E], F32)
                nc.vector.tensor_add(sb_[:], sig[:], gate_b_sb[:])
                mx8 = pd.tile([P, 8], F32)
                nc.vector.max(out=mx8[:], in_=sb_[:])
                s1 = pd.tile([P, E], F32)
                nc.vector.tensor_tensor(out=s1[:], in0=sb_[:],
                                        in1=mx8[:, 0:1].to_broadcast([P, E]),
                                        op=ALU.is_equal)
                s2 = pd.tile([P, E], F32)
                nc.vector.tensor_tensor(out=s2[:], in0=sb_[:],
                                        in1=mx8[:, 1:2].to_broadcast([P, E]),
                                        op=ALU.is_equal)
                nc.vector.tensor_add(s1[:], s1[:], s2[:])
                nc.vector.tensor_scalar_min(s1[:], s1[:], 1.0)
                wa = pd.tile([P, E], F32)
                nc.vector.tensor_mul(wa[:], s1[:], sig[:])
                nrm = pd.tile([P, 1], F32)
                nc.vector.reduce_sum(nrm[:], wa[:], axis=AX.X)
                rec = pd.tile([P, 1], F32)
                nc.vector.reciprocal(rec[:], nrm[:])
                paw = pd.tile([P, WP], F16)
                nc.vector.tensor_scalar_mul(paw[:, 0:E], wa[:], rec[:, :1])
                selh = pd.tile([P, E], F16)
                nc.vector.tensor_copy(selh[:], s1[:])
                uml = pd.tile([P, E], F32)
                nc.vector.tensor_scalar(out=uml[:], in0=selh[:],
                                        scalar1=-BIG2, scalar2=BIG2,
                                        op0=ALU.mult, op1=ALU.add)
                pre_l = pd.tile([P, E], F16)
                with tc.tile_pool(name="psDp", bufs=1, space="PSUM") as psDp:
                    prp = psDp.tile([P, E], F32, tag="prp")
                    nc.tensor.matmul(prp[:], ut_sb[:], selh[:],
                                     start=True, stop=True)
                    nc.vector.tensor_copy(pre_l[:], prp[:])
                    nc.vector.tensor_tensor(out=paw[:, E:2 * E], in0=prp[:],
                                            in1=uml[:], op=ALU.add)
                    tbp = psDp.tile([P, E], F32, tag="tbp")
                    nc.tensor.matmul(tbp[:], bc127[:], pre_l[:],
                                     start=True, stop=True)
                    nc.vector.tensor_copy(paw[:, 2 * E:3 * E], tbp[:])
                nc.scalar.dma_start(agx1_in[:, H // 2:H // 2 + WP], paw[:])

            nc.gpsimd.collective_compute(
                "AllGather", ALU.bypass, ins=[agx1_in.opt()],
                outs=[x_tmA.opt()], replica_groups=RG)
            nc.gpsimd.collective_compute(
                "AllGather", ALU.bypass, ins=[agx2_in.opt()],
                outs=[x_tmB.opt()], replica_groups=RG)

            # ======== Phase X: token lists + x^T + shared + experts ======
            with tc.tile_pool(name="pg", bufs=1) as pg, \
                 tc.tile_pool(name="pg2", bufs=2) as pg2:
                xT = [pg.tile([P, T], F16, tag=f"xT{hc}", name=f"xT{hc}")
                      for hc in range(HC)]
                totals = pg.tile([8, E], F16)
                grank = [pg.tile([P, E], F32, tag=f"grank{b}",
                                 name=f"grank{b}") for b in range(TB)]
                wb0 = H // 2
                with tc.tile_pool(name="psXr", bufs=2, space="PSUM") as psXr:
                    nc.sync.dma_start(
                        totals[:].rearrange("b (o e) -> b o e", o=1),
                        x_tmA[:].rearrange("(b p) e -> b p e", p=P)[
                            :, 0:1, wb0 + 2 * E:wb0 + 3 * E])
                    wrbs = []
                    for b in range(TB):
                        wrb = pg2.tile([P, WP], F16, tag="wrb", bufs=8,
                                       name=f"wrb{b}")
                        nc.sync.dma_start(
                            wrb[:], x_tmA[b * P:(b + 1) * P, wb0:wb0 + WP])
                        wrbs.append(wrb)
                    for b in range(TB):
                        ofs = psXr.tile([P, E], F32, tag="ofs")
                        nc.tensor.matmul(ofs[:], slb_sb[:, b * P:(b + 1) * P],
                                         totals[:], start=True, stop=True)
                        nc.vector.tensor_tensor(out=grank[b][:],
                                                in0=wrbs[b][:, E:2 * E],
                                                in1=ofs[:], op=ALU.add)
                        gm = pg2.tile([P, E], F32, tag="gm")
                        nc.vector.tensor_scalar(out=gm[:], in0=grank[b][:],
                                                scalar1=float(CAP),
                                                scalar2=BIG,
                                                op0=ALU.is_gt, op1=ALU.mult)
                        nc.vector.tensor_add(grank[b][:], grank[b][:], gm[:])
                    sent = pg.tile([P, 1], I32)
                    nc.vector.memset(sent[:], 1000000)
                    for kk in range(2 * SL // P):
                        nc.sync.dma_start(tok_lists[kk * P:(kk + 1) * P, :],
                                          sent[:])
                    for b in range(TB):
                        tok = pg2.tile([P, 1], I32, tag="tok")
                        nc.gpsimd.iota(tok[:], pattern=[[0, 1]], base=b * P,
                                       channel_multiplier=1)
                        for ei in range(2):
                            gsel = pg2.tile([P, E], F32, tag="gsel")
                            nc.vector.tensor_mul(gsel[:], grank[b][:],
                                                 em[ei][:])
                            ridx = pg2.tile([P, 1], F32, tag="ridx")
                            nc.vector.reduce_sum(ridx[:], gsel[:], axis=AX.X)
                            nc.vector.tensor_scalar_add(ridx[:], ridx[:],
                                                        float(ei * SL - 1))
                            ridx_i = pg2.tile([P, 1], I32, tag="ridxi")
                            nc.vector.tensor_copy(ridx_i[:], ridx[:])
                            nc.gpsimd.indirect_dma_start(
                                out=tok_lists[:],
                                out_offset=bass.IndirectOffsetOnAxis(
                                    ap=ridx_i[:, :1], axis=0),
                                in_=tok[:], in_offset=None,
                                bounds_check=2 * SL - 1, oob_is_err=False)

                # x^T build: half A (hc 0-7) then half B (hc 8-15),
                # shared-expert gate pass interleaved
                hsh = [pg.tile([P, T], F16, tag=f"hs{sp}", name=f"hs{sp}")
                       for sp in range(2)]
                gsh = [pg.tile([P, T], F16, tag=f"gsh{sp}", name=f"gsh{sp}")
                       for sp in range(2)]
                with tc.tile_pool(name="psXt", bufs=2, space="PSUM") as psXt, \
                     tc.tile_pool(name="psS", bufs=1, space="PSUM") as psS:
                    gps = [psS.tile([P, T], F32, tag=f"sgp{sp}",
                                    name=f"sgp{sp}") for sp in range(2)]
                    for half, src_tm, hclo in [(0, x_tmB, 0), (1, x_tmA, 8)]:
                        for n in range(2):
                            xbs = []
                            for bb in range(4):
                                b = n * 4 + bb
                                xb = pg2.tile([P, H // 2], F16, tag="xb",
                                              bufs=4, name=f"xb{half}_{b}")
                                eng = nc.sync if bb % 2 == 0 else nc.scalar
                                eng.dma_start(
                                    xb[:], src_tm[b * P:(b + 1) * P,
                                                  0:H // 2])
                                xbs.append(xb)
                            sl = slice(n * 512, (n + 1) * 512)
                            for hc8 in range(8):
                                hc = hclo + hc8
                                tp = psXt.tile([P, 512], F16, tag="tpX")
                                for bb in range(4):
                                    nc.tensor.transpose(
                                        tp[:, bb * P:(bb + 1) * P],
                                        xbs[bb][:, hc8 * P:(hc8 + 1) * P],
                                        identh[:])
                                if hc % 2 == 0:
                                    nc.vector.tensor_copy(xT[hc][:, sl], tp[:])
                                else:
                                    nc.scalar.activation(xT[hc][:, sl], tp[:],
                                                         AF.Copy)
                        # shared gate pass for this half's hc range
                        for hc8 in range(8):
                            hc = hclo + hc8
                            for sp in range(2):
                                c0 = hc * ISC + sp * P
                                for nn in range(2):
                                    sl = slice(nn * 512, (nn + 1) * 512)
                                    nc.tensor.matmul(gps[sp][:, sl],
                                                     wsg_sb[:, c0:c0 + P],
                                                     xT[hc][:, sl],
                                                     start=(hc == 0),
                                                     stop=(hc == HC - 1))
                    for sp in range(2):
                        nc.vector.tensor_copy(gsh[sp][:], gps[sp][:])
                        nc.scalar.activation(gsh[sp][:], gsh[sp][:], AF.Silu)
                    ups = [psS.tile([P, T], F32, tag=f"sgp{sp}",
                                    name=f"sup{sp}") for sp in range(2)]
                    for hc in range(HC):
                        for sp in range(2):
                            c0 = hc * ISC + sp * P
                            for nn in range(2):
                                sl = slice(nn * 512, (nn + 1) * 512)
                                nc.tensor.matmul(ups[sp][:, sl],
                                                 wsu_sb[:, c0:c0 + P],
                                                 xT[hc][:, sl],
                                                 start=(hc == 0),
                                                 stop=(hc == HC - 1))
                    for sp in range(2):
                        nc.vector.tensor_mul(hsh[sp][:], gsh[sp][:],
                                             ups[sp][:])
                with tc.tile_pool(name="psS3", bufs=2, space="PSUM") as psS3:
                    for nn in range(2):
                        for tb2 in range(TB // 2):
                            sd = pg2.tile([P, 2048], F16, tag="sd", bufs=1)
                            for two in range(2):
                                tb_ = tb2 * 2 + two
                                ps3 = psS3.tile([P, 1024], F32, tag="psSd")
                                for sp in range(2):
                                    for q2 in range(2):
                                        s2 = slice(q2 * 512, (q2 + 1) * 512)
                                        nc.tensor.matmul(
                                            ps3[:, s2],
                                            hsh[sp][:, tb_ * P:(tb_ + 1) * P],
                                            wsd_sb[sp][:, nn * 1024 + q2 * 512:
                                                        nn * 1024 + (q2 + 1) * 512],
                                            start=(sp == 0), stop=(sp == 1))
                                dst = sd[:, two * 1024:(two + 1) * 1024]
                                if two == 0:
                                    nc.vector.tensor_copy(dst, ps3[:])
                                else:
                                    nc.scalar.activation(dst, ps3[:], AF.Copy)
                            eng = nc.sync if tb2 % 2 == 0 else nc.scalar
                            eng.dma_start(
                                rs2_in[nn][tb2 * 2 * P:(tb2 + 1) * 2 * P, :]
                                .rearrange("(two p) c -> p two c", two=2),
                                sd[:].rearrange("p (two c) -> p two c", two=2))

                # ======== Expert gathers + wcol (gpsimd ahead of PE) ======
                KL = [P, CAP - P]
                idx_sb = [[pg.tile([P if k == 0 else CAP - P, 1], I32,
                                   tag=f"idx{ei}_{k}",
                                   name=f"idx{ei}_{k}") for k in range(2)]
                          for ei in range(2)]
                gxT = [pg.tile([P, HC * SL], F16, tag=f"gxT{ei}",
                               name=f"gxT{ei}") for ei in range(2)]
                wcol = [[pg.tile([P if k == 0 else CAP - P, 1], F32,
                                 tag=f"wcol{ei}_{k}",
                                 name=f"wcol{ei}_{k}") for k in range(2)]
                        for ei in range(2)]
                with tc.tile_pool(name="psFt", bufs=2, space="PSUM") as psFt:
                    for ei in range(2):
                        gxA = [None, None]
                        gxB = [None, None]
                        for k in range(2):
                            nc.sync.dma_start(
                                idx_sb[ei][k][:],
                                tok_lists[ei * SL + k * P:
                                          ei * SL + k * P + KL[k], :])
                            ga_ = pg2.tile([P, H // 2 + WP], F16, tag="gxA",
                                           name=f"gxA{ei}_{k}")
                            nc.vector.memset(ga_[:KL[k], :], 0.0)
                            nc.gpsimd.indirect_dma_start(
                                out=ga_[:KL[k], :], out_offset=None,
                                in_=x_tmA[:],
                                in_offset=bass.IndirectOffsetOnAxis(
                                    ap=idx_sb[ei][k][:, :1], axis=0),
                                bounds_check=T - 1, oob_is_err=False)
                            gb_ = pg2.tile([P, H // 2], F16, tag="gxB",
                                           name=f"gxB{ei}_{k}")
                            nc.vector.memset(gb_[:KL[k], :], 0.0)
                            nc.gpsimd.indirect_dma_start(
                                out=gb_[:KL[k], :], out_offset=None,
                                in_=x_tmB[:],
                                in_offset=bass.IndirectOffsetOnAxis(
                                    ap=idx_sb[ei][k][:, :1], axis=0),
                                bounds_check=T - 1, oob_is_err=False)
                            gxA[k] = ga_
                            gxB[k] = gb_
                        # wcol first (vector-only, unblocks nothing behind)
                        for k in range(2):
                            wtmp_f = pg2.tile([P, E], F32, tag="wtmp")
                            wtmp = wtmp_f[:KL[k], :]
                            nc.vector.tensor_mul(
                                wtmp, gxA[k][:KL[k], wb0:wb0 + E],
                                em[ei][:KL[k], :])
                            nc.vector.reduce_sum(wcol[ei][k][:], wtmp,
                                                 axis=AX.X)
                        for hp in range(HC // 2):
                            tp = psFt.tile([P, 2 * CAP], F16, tag="tpF")
                            for i in range(2):
                                hc = hp * 2 + i
                                o0 = i * CAP
                                gsrc = gxB if hc < 8 else gxA
                                c0 = (hc % 8) * P
                                nc.tensor.transpose(
                                    tp[:, o0:o0 + P],
                                    gsrc[0][:, c0:c0 + P],
                                    identh[:])
                                nc.tensor.transpose(
                                    tp[:, o0 + P:o0 + CAP],
                                    gsrc[1][:KL[1], c0:c0 + P],
                                    identh[:KL[1], :KL[1]])
                            dst = gxT[ei][:].rearrange(
                                "p (hc c) -> p hc c", hc=HC)[
                                :, hp * 2:hp * 2 + 2, 0:CAP]
                            src = tp[:].rearrange("p (hc c) -> p hc c", hc=2)
                            if hp % 2 == 0:
                                nc.vector.tensor_copy(dst, src)
                            else:
                                nc.scalar.activation(dst, src, AF.Copy)

                # ======== Experts: I-partitioned gate/up ======
                h_sb = [pg.tile([P, IP * CAP], F16, tag=f"h_sb{ei}",
                                name=f"h_sb{ei}") for ei in range(2)]
                sg_sb = pg.tile([P, IP * CAP], F16)
                wd_res = [pg.tile([P, H], F16, tag=f"wd{e}_{ip}",
                                  name=f"wd{e}_{ip}")
                          for e in range(2) for ip in range(IP)]
                for ip in range(IP):
                    nc.gpsimd.dma_start(wd_res[ip][:],
                                        ex["we_d"][0, ip * P:(ip + 1) * P, :])
                wi = 2
                for ei in range(2):
                    if ei == 1:
                        for ip in range(IP):
                            nc.gpsimd.dma_start(
                                wd_res[IP + ip][:],
                                ex["we_d"][1, ip * P:(ip + 1) * P, :])
                    with tc.tile_pool(name=f"psF1{ei}", bufs=1,
                                      space="PSUM") as psF1:
                        acc = [psF1.tile([P, 256], F32, tag=f"acc{ip}",
                                         name=f"acc{ip}_{ei}")
                               for ip in range(IP)]
                        for kind in ("g", "u"):
                            base = (0 if kind == "g" else NJ) + ei * 2 * NJ
                            for j in range(NJ):
                                wp = wring[base + j]
                                if wi < len(wseq):
                                    issue_pair(wi)
                                    wi += 1
                                for four in range(4):
                                    hc = 4 * j + four
                                    for ip in range(IP):
                                        nc.tensor.matmul(
                                            acc[ip][:, :CAP],
                                            wp[:, four * I + ip * P:
                                               four * I + (ip + 1) * P],
                                            gxT[ei][:, hc * SL:hc * SL + CAP],
                                            start=(hc == 0),
                                            stop=(hc == HC - 1))
                            if kind == "g":
                                for ip in range(IP):
                                    nc.scalar.activation(
                                        sg_sb[:, ip * CAP:(ip + 1) * CAP],
                                        acc[ip][:, :CAP], AF.Silu)
                            else:
                                for ip in range(IP):
                                    nc.vector.tensor_mul(
                                        h_sb[ei][:, ip * CAP:(ip + 1) * CAP],
                                        sg_sb[:, ip * CAP:(ip + 1) * CAP],
                                        acc[ip][:, :CAP])

                # ======== Down-projections, column-half outer + RS2 ======
                with tc.tile_pool(name="psF3", bufs=2, space="PSUM") as psF3:
                    for nn in range(2):
                        for ei in range(2):
                            for k in range(2):
                                kl = KL[k]
                                koff = k * P
                                psd = psF3.tile([P, 1024], F32, tag="fd")
                                for ip in range(IP):
                                    c0 = ip * CAP + koff
                                    for q2 in range(2):
                                        s2 = slice(q2 * 512, (q2 + 1) * 512)
                                        nc.tensor.matmul(
                                            psd[:kl, s2],
                                            h_sb[ei][:, c0:c0 + kl],
                                            wd_res[ei * IP + ip][
                                                :, nn * 1024 + q2 * 512:
                                                nn * 1024 + (q2 + 1) * 512],
                                            start=(ip == 0),
                                            stop=(ip == IP - 1))
                                out_f = pg2.tile([P, 1024], F16, tag="outsb",
                                                 name=f"outsb{nn}{ei}{k}")
                                nc.vector.tensor_scalar_mul(
                                    out_f[:kl, :], psd[:kl, :],
                                    wcol[ei][k][:, :1])
                                nc.gpsimd.indirect_dma_start(
                                    out=rs2_in[nn][:],
                                    out_offset=bass.IndirectOffsetOnAxis(
                                        ap=idx_sb[ei][k][:, :1], axis=0),
                                    in_=out_f[:kl, :], in_offset=None,
                                    bounds_check=T - 1, oob_is_err=False,
                                    compute_op=ALU.add)
                        nc.gpsimd.collective_compute(
                            "ReduceScatter", ALU.add, ins=[rs2_in[nn].opt()],
                            outs=[rs2_out[nn].opt()], replica_groups=RG)

            for nn in range(2):
                nc.sync.dma_start(out_slice[:, nn * 1024:(nn + 1) * 1024],
                                  rs2_out[nn][:])


_CACHE = {}


def _build():
    key = "nc"
    if key in _CACHE:
        return _CACHE[key]
    nc = bacc.Bacc("TRN2", target_bir_lowering=False, debug=False,
                   num_devices=NCN)
    with tile.TileContext(nc) as tc:
        _emit(nc, tc)
    nc.compile()
    _CACHE[key] = nc
    return nc


def _perm_rows(c):
    return np.concatenate([np.arange(q * CH + c * SH, q * CH + (c + 1) * SH)
                           for q in range(NCH)])


def _host_prep(inputs):
    f16 = np.float16
    pos = np.asarray(inputs["positions"]).astype(np.float64)
    hid = np.asarray(inputs["hidden_states"], np.float32)
    w_in = np.asarray(inputs["w_in_ln"], np.float32)
    w_post = np.asarray(inputs["w_post_ln"], np.float32)
    wq = np.asarray(inputs["wq"], np.float32) * w_in[:, None]
    wk = np.asarray(inputs["wk"], np.float32) * w_in[:, None]
    wv = np.asarray(inputs["wv"], np.float32) * w_in[:, None]
    wo = np.asarray(inputs["wo"], np.float32)
    gate_w = np.asarray(inputs["gate_w"], np.float32) * w_post[None, :]
    gate_b = np.asarray(inputs["gate_bias"], np.float32).reshape(1, E)
    we_g = (np.asarray(inputs["we_gate"], np.float32)
            * w_post[None, :, None]).astype(f16)
    we_u = (np.asarray(inputs["we_up"], np.float32)
            * w_post[None, :, None]).astype(f16)
    we_d = np.asarray(inputs["we_down"], np.float32).astype(f16)
    ws_g = np.asarray(inputs["ws_gate"], np.float32) * w_post[:, None]
    ws_u = np.asarray(inputs["ws_up"], np.float32) * w_post[:, None]
    ws_d = np.asarray(inputs["ws_down"], np.float32).astype(f16)

    inv_freq = 1.0 / (THETA ** (np.arange(0, D, 2, dtype=np.float64) / D))
    f = pos[None, :] * inv_freq[:, None]
    cos2, sin2 = np.cos(f), np.sin(f)
    cosT = np.repeat(cos2, 2, axis=0).astype(np.float32)
    sinT = np.empty((D, T), np.float32)
    sinT[0::2] = -sin2
    sinT[1::2] = sin2
    s = 1.0 / np.sqrt(D)
    cosq, sinq = (cosT * s).astype(f16), (sinT * s).astype(f16)
    cosk, sink = cosT.astype(f16), sinT.astype(f16)

    ii = np.arange(P)
    diagmask = np.where(ii[:, None] >= ii[None, :], 0.0, NEG).astype(f16)
    ident = np.eye(P, dtype=np.float32)
    ut_in = np.triu(np.ones((P, P), np.float32)).astype(f16)
    slb_in = np.zeros((8, TB * P), np.float32)
    for b in range(TB):
        slb_in[:b, b * P:(b + 1) * P] = 1.0
    slb_in = slb_in.astype(f16)
    bc127 = np.zeros((P, P), np.float32)
    bc127[127, :] = 1.0
    bc127 = bc127.astype(f16)
    perm = np.zeros((P, P), np.float32)
    for i in range(0, P, 2):
        perm[i, i + 1] = 1.0
        perm[i + 1, i] = 1.0

    def pack_pk(w, width):  # w: [H, width]
        return np.ascontiguousarray(
            w.reshape(HC, P, width).transpose(1, 0, 2).reshape(P, HC * width))

    gate_w_pk = pack_pk(gate_w.T.astype(np.float32), E)

    maps = []
    for c in range(NCN):
        g = c // 2
        w_qkv = pack_pk(np.concatenate([
            wq[:, 2 * c * D:(2 * c + 1) * D],
            wq[:, (2 * c + 1) * D:(2 * c + 2) * D],
            wk[:, g * D:(g + 1) * D],
            wv[:, g * D:(g + 1) * D]], axis=1), 512).astype(f16)
        em0 = np.zeros((P, E), np.float32)
        em0[:, 2 * c] = 1.0
        em1 = np.zeros((P, E), np.float32)
        em1[:, 2 * c + 1] = 1.0
        maps.append({
            "hid": hid.astype(f16),
            "hid_slice": np.ascontiguousarray(hid[_perm_rows(c)]),
            "w_qkv_pk": w_qkv,
            "wo0": np.ascontiguousarray(wo[2 * c * D:(2 * c + 1) * D]).astype(f16),
            "wo1": np.ascontiguousarray(
                wo[(2 * c + 1) * D:(2 * c + 2) * D]).astype(f16),
            "cosq": cosq, "sinq": sinq, "cosk": cosk, "sink": sink,
            "permh": perm.astype(f16), "identh_in": ident.astype(f16),
            "identr_in": ident, "diagmask": diagmask,
            "gate_w_pk": gate_w_pk,
            "gate_b": np.broadcast_to(gate_b, (P, E)).astype(np.float32).copy(),
            "emask0": em0, "emask1": em1,
            "ut_in": ut_in, "slb_in": slb_in, "bcast127": bc127,
            "ws_g_pk": pack_pk(
                ws_g[:, c * ISC:(c + 1) * ISC].astype(np.float32), ISC
            ).astype(f16),
            "ws_u_pk": pack_pk(
                ws_u[:, c * ISC:(c + 1) * ISC].astype(np.float32), ISC
            ).astype(f16),
            "ws_d": np.ascontiguousarray(ws_d[c * ISC:(c + 1) * ISC]),
            "we_g": np.ascontiguousarray(we_g[2 * c:2 * c + 2]),
            "we_u": np.ascontiguousarray(we_u[2 * c:2 * c + 2]),
            "we_d": np.ascontiguousarray(we_d[2 * c:2 * c + 2]),
        })
    return maps


def kernel(trace=False, **inputs):
    nc = _build()
    maps = _host_prep(inputs)
    res = bass_utils.run_bass_kernel_spmd(
        nc, maps, core_ids=list(range(NCN)), trace=trace)
    out = np.empty((T, H), np.float32)
    resid = np.empty((T, H), np.float32)
    for c in range(NCN):
        rows = _perm_rows(c)
        out[rows] = res.results[c]["out_slice"].astype(np.float32)
        resid[rows] = res.results[c]["res_slice"]
    kernel.last_results = res
    return out, resid


# revision 30
# speedup vs baseline: 1.0952x; 1.0247x over previous
"""Ernie4 decoder layer (RMSNorm + GQA attention + shared expert + 16-expert
top-2 MoE) on 8 Trainium2 NeuronCores.

v7 (pipelined collectives):
  - Attention head-parallel, processed query-block-major: per 128-token
    block both heads' scores/softmax/AV and the o_proj run immediately,
    feeding 2 token-chunked ReduceScatters that fire DURING attention.
    Token ownership becomes permuted (32-row shards per chunk); the host
    permutes hid_slice in and unpermutes outputs.
  - x AllGather splits in two column chunks; collectives execute in
    readiness order, so the early chunk carries the x lo-half (ready at
    the norm) and the late chunk carries [x hi-half | router payload]
    (ready after the router). x^T build and the shared-expert gate pass
    consume the halves in arrival order.
  - Expert capacity 160 (max actual load ~155); gate/up I-partitioned
    (no h transposes); weight-scale (wcol) gathers run before expert
    compute so the gpsimd queue never blocks the down-projections.
  - MoE/shared/router weights prefetch on the gpsimd software-DGE queues
    during attention; expert weights stream as 1MB pairs alternating the
    two HWDGE queues; down-projections run column-half-outer feeding 2
    chunked ReduceScatters so RS2a overlaps the second half's compute.
"""
import sys
sys.path.insert(0, "/opt/trn_rl_repo")

import numpy as np

import concourse.bass as bass
import concourse.bacc as bacc
import concourse.tile as tile
import concourse.mybir as mybir
from concourse import bass_utils

dt = mybir.dt
F32 = dt.float32
F16 = dt.float16
I32 = dt.int32
AF = mybir.ActivationFunctionType
ALU = mybir.AluOpType
AX = mybir.AxisListType

T, H, NH, NKV, D = 1024, 2048, 16, 4, 128
E, I, IS = 16, 1024, 2048
ISC = IS // 8
EPS = 1e-6
THETA = 10000.0
NCN = 8
P = 128
TB = T // P
HC = H // P
IP = I // P
CAP = 160               # per-expert compute capacity (rank mask)
SL = 256                # per-expert list-slot spacing (square layouts)
NCH = 2                 # RS1 token chunks
CH = T // NCH           # 256 tokens per chunk
SH = CH // NCN          # 32-row per-core shard per chunk
WP = 3 * E              # router payload width
BIG = 1.0e6
BIG2 = 30000.0
NEG = -30000.0
RG = [list(range(NCN))]


def _emit(nc, tc):
    ex = {}
    for name, shape, d in [
        ("hid", [T, H], F16), ("hid_slice", [P, H], F32),
        ("w_qkv_pk", [P, HC * 512], F16),
        ("wo0", [D, H], F16), ("wo1", [D, H], F16),
        ("cosq", [D, T], F16), ("sinq", [D, T], F16),
        ("cosk", [D, T], F16), ("sink", [D, T], F16),
        ("permh", [P, P], F16), ("identh_in", [P, P], F16),
        ("identr_in", [P, P], F32), ("diagmask", [P, P], F16),
        ("gate_w_pk", [P, HC * E], F32), ("gate_b", [P, E], F32),
        ("emask0", [P, E], F32), ("emask1", [P, E], F32),
        ("ut_in", [P, P], F16), ("slb_in", [8, TB * P], F16),
        ("bcast127", [P, P], F16),
        ("ws_g_pk", [P, HC * ISC], F16), ("ws_u_pk", [P, HC * ISC], F16),
        ("ws_d", [ISC, H], F16),
        ("we_g", [2, H, I], F16), ("we_u", [2, H, I], F16),
        ("we_d", [2, I, H], F16),
    ]:
        ex[name] = nc.dram_tensor(name, shape, d, kind="ExternalInput").ap()
    out_slice = nc.dram_tensor("out_slice", [P, H], F16, kind="ExternalOutput").ap()
    res_slice = nc.dram_tensor("res_slice", [P, H], F32, kind="ExternalOutput").ap()

    with tc.tile_pool(name="pp", bufs=1) as pp, \
         tc.tile_pool(name="dram", bufs=1, space="DRAM") as dram:
        rs1_in = [dram.tile([CH, H], F16, tag=f"rs1i{q}", name=f"rs1i{q}")
                  for q in range(NCH)]
        rs1_out = [dram.tile([SH, H], F16, tag=f"rs1o{q}", name=f"rs1o{q}")
                   for q in range(NCH)]
        agx1_in = dram.tile([P, H // 2 + WP], F16)
        agx2_in = dram.tile([P, H // 2], F16)
        x_tmA = dram.tile([T, H // 2 + WP], F16, addr_space="Shared")
        x_tmB = dram.tile([T, H // 2], F16, addr_space="Shared")
        tok_lists = dram.tile([2 * SL, 1], I32)
        rs2_in = [dram.tile([T, H // 2], F16, tag=f"rs2i{nn}",
                            name=f"rs2i{nn}") for nn in range(2)]
        rs2_out = [dram.tile([P, H // 2], F16, tag=f"rs2o{nn}",
                             name=f"rs2o{nn}") for nn in range(2)]

        identh = pp.tile([P, P], F16)
        nc.sync.dma_start(identh[:], ex["identh_in"][:])
        identf = pp.tile([P, P], F32)
        nc.sync.dma_start(identf[:], ex["identr_in"][:])
        eps_t = pp.tile([P, 1], F32)
        nc.vector.memset(eps_t[:], EPS)

        # ======== persistent weight pool (prefetched during attention) ====
        with tc.tile_pool(name="pw", bufs=1) as pw, \
             tc.tile_pool(name="pfw", bufs=1) as pfw:
            wsg_sb = pw.tile([P, HC * ISC], F16)
            wsu_sb = pw.tile([P, HC * ISC], F16)
            wsd_sb = [pw.tile([P, H], F16, tag=f"wsd{sp}", name=f"wsd{sp}")
                      for sp in range(2)]
            gwr = pw.tile([P, HC * E], F32)
            gate_b_sb = pw.tile([P, E], F32)
            ut_sb = pw.tile([P, P], F16)
            bc127 = pw.tile([P, P], F16)
            slb_sb = pw.tile([8, TB * P], F16)
            em = [pw.tile([P, E], F32, tag=f"em{e}", name=f"em{e}")
                  for e in range(2)]

            NJ = IP // 2  # 4 hc rows per 1MB pair load
            wseq = [(k, ei, j) for ei in range(2) for k in ("g", "u")
                    for j in range(NJ)]
            wring = {}
            WIN = 3

            def issue_pair(i):
                k, ei, j = wseq[i]
                src = ex["we_g"] if k == "g" else ex["we_u"]
                t_ = pfw.tile([P, 4 * I], F16, tag="wp", bufs=WIN,
                              name=f"wp{i}")
                eng = nc.sync if i % 2 == 0 else nc.scalar
                eng.dma_start(
                    t_[:].rearrange("p (four i) -> p four i", four=4),
                    src[ei, j * 4 * P:(j + 1) * 4 * P, :].rearrange(
                        "(four a) i -> a four i", a=P))
                wring[i] = t_

            # ======== Phase A: norm + transpose + QKV + rope ========
            with tc.tile_pool(name="pab", bufs=1) as pab:
                qT = [pab.tile([P, T], F16, tag=f"qT{j}", name=f"qT{j}")
                      for j in range(2)]
                kT = pab.tile([P, T], F16)
                v_tm = pab.tile([P, TB * D], F16)
                wo_sb = [pab.tile([P, H], F16, tag=f"wo{j}", name=f"wo{j}")
                         for j in range(2)]
                diagm = pab.tile([P, P], F16)

                with tc.tile_pool(name="pa", bufs=1) as pa, \
                     tc.tile_pool(name="pa2", bufs=2) as pa2:
                    hidbs = []
                    for b in range(TB):
                        t_ = pa2.tile([P, H], F16, tag="hidb", bufs=8,
                                      name=f"hidb{b}")
                        nc.sync.dma_start(t_[:], ex["hid"][b * P:(b + 1) * P, :])
                        hidbs.append(t_)
                    nc.sync.dma_start(wo_sb[0][:], ex["wo0"][:])
                    nc.sync.dma_start(wo_sb[1][:], ex["wo1"][:])
                    nc.sync.dma_start(diagm[:], ex["diagmask"][:])
                    cosq = pa.tile([D, T], F16)
                    sinq = pa.tile([D, T], F16)
                    cosk = pa.tile([D, T], F16)
                    sink = pa.tile([D, T], F16)
                    for t_, s_ in [(cosq, "cosq"), (sinq, "sinq"),
                                   (cosk, "cosk"), (sink, "sink")]:
                        nc.gpsimd.dma_start(t_[:], ex[s_][:])
                    permh = pa.tile([P, P], F16)
                    nc.gpsimd.dma_start(permh[:], ex["permh"][:])
                    wqkv_sb = pa.tile([P, HC * 512], F16)
                    nc.sync.dma_start(wqkv_sb[:], ex["w_qkv_pk"][:])
                    # persistent-weight prefetch (runs during attention)
                    nc.gpsimd.dma_start(wsg_sb[:], ex["ws_g_pk"][:])
                    nc.gpsimd.dma_start(wsu_sb[:], ex["ws_u_pk"][:])
                    for sp in range(2):
                        nc.gpsimd.dma_start(wsd_sb[sp][:],
                                            ex["ws_d"][sp * P:(sp + 1) * P, :])
                    nc.gpsimd.dma_start(gwr[:], ex["gate_w_pk"][:])
                    nc.gpsimd.dma_start(gate_b_sb[:], ex["gate_b"][:])
                    nc.gpsimd.dma_start(ut_sb[:], ex["ut_in"][:])
                    nc.gpsimd.dma_start(bc127[:], ex["bcast127"][:])
                    nc.gpsimd.dma_start(slb_sb[:], ex["slb_in"][:])
                    nc.gpsimd.dma_start(em[0][:], ex["emask0"][:])
                    nc.gpsimd.dma_start(em[1][:], ex["emask1"][:])
                    for i in range(2):
                        issue_pair(i)

                    x0T = [pa.tile([P, T], F16, tag=f"x0T{hc}",
                                   name=f"x0T{hc}") for hc in range(HC)]
                    qraw = [pa.tile([P, T], F16, tag=f"qraw{j}",
                                    name=f"qraw{j}") for j in range(2)]
                    kraw = pa.tile([P, T], F16)
                    vraw = pa.tile([P, T], F16)
                    dump = pa.tile([P, H], F32)

                    with tc.tile_pool(name="psA1", bufs=2, space="PSUM") as psA1, \
                         tc.tile_pool(name="psA2", bufs=1, space="PSUM") as psA2:
                        for n in range(2):
                            x0hs = []
                            for bb in range(TB // 2):
                                b = n * (TB // 2) + bb
                                hidb = hidbs[b]
                                ssum = pa2.tile([P, 1], F32, tag="ssum")
                                nc.scalar.activation(dump[:], hidb[:],
                                                     AF.Square,
                                                     accum_out=ssum[:, :1])
                                rms = pa2.tile([P, 1], F32, tag="rms")
                                nc.scalar.activation(rms[:], ssum[:], AF.Sqrt,
                                                     bias=eps_t[:, :1],
                                                     scale=1.0 / H)
                                inv = pa2.tile([P, 1], F32, tag="inv")
                                nc.vector.reciprocal(inv[:], rms[:])
                                x0h = pa2.tile([P, H], F16, tag="x0h", bufs=4,
                                               name=f"x0h{b}")
                                nc.vector.tensor_scalar_mul(x0h[:], hidb[:],
                                                            inv[:, :1])
                                x0hs.append(x0h)
                            sl = slice(n * 512, (n + 1) * 512)
                            for hc in range(HC):
                                tp = psA1.tile([P, 512], F16, tag="tpA")
                                for bb in range(4):
                                    nc.tensor.transpose(
                                        tp[:, bb * P:(bb + 1) * P],
                                        x0hs[bb][:, hc * P:(hc + 1) * P],
                                        identh[:])
                                if hc % 2 == 0:
                                    nc.vector.tensor_copy(x0T[hc][:, sl], tp[:])
                                else:
                                    nc.scalar.activation(x0T[hc][:, sl], tp[:],
                                                         AF.Copy)
                            ps4 = [psA2.tile([P, 512], F32, tag=f"qkv{j}",
                                             name=f"qkv{j}_{n}")
                                   for j in range(4)]
                            for hc in range(HC):
                                for j, c0 in enumerate([0, 128, 256, 384]):
                                    nc.tensor.matmul(
                                        ps4[j][:],
                                        wqkv_sb[:, hc * 512 + c0:
                                                hc * 512 + c0 + P],
                                        x0T[hc][:, sl],
                                        start=(hc == 0), stop=(hc == HC - 1))
                            for j, dst in [(2, kraw), (3, vraw),
                                           (0, qraw[0]), (1, qraw[1])]:
                                if j % 2 == 0:
                                    nc.vector.tensor_copy(dst[:, sl], ps4[j][:])
                                else:
                                    nc.scalar.activation(dst[:, sl], ps4[j][:],
                                                         AF.Copy)

                    with tc.tile_pool(name="psA3", bufs=2, space="PSUM") as psA3, \
                         tc.tile_pool(name="psA4", bufs=2, space="PSUM") as psA4:
                        for src, dst, c_, s_ in [(kraw, kT, cosk, sink),
                                                 (qraw[0], qT[0], cosq, sinq),
                                                 (qraw[1], qT[1], cosq, sinq)]:
                            sw = psA3.tile([P, T], F32, tag="sw")
                            for nn in range(2):
                                sl = slice(nn * 512, (nn + 1) * 512)
                                nc.tensor.matmul(sw[:, sl], permh[:], src[:, sl],
                                                 start=True, stop=True)
                            t1 = pa2.tile([P, T], F16, tag="ropet1")
                            nc.gpsimd.tensor_mul(t1[:], src[:], c_[:])
                            t2 = pa2.tile([P, T], F16, tag="ropet2")
                            nc.vector.tensor_mul(t2[:], sw[:], s_[:])
                            nc.gpsimd.tensor_add(dst[:], t1[:], t2[:])
                        for g4 in range(2):
                            tp = psA4.tile([P, 512], F16, tag="tpV")
                            for bb in range(4):
                                b = g4 * 4 + bb
                                nc.tensor.transpose(
                                    tp[:, bb * P:(bb + 1) * P],
                                    vraw[:, b * P:(b + 1) * P], identh[:])
                            nc.vector.tensor_copy(
                                v_tm[:, g4 * 512:(g4 + 1) * 512], tp[:])

                # ==== Phase B: per-block attention + o_proj + chunked RS1 ==
                with tc.tile_pool(name="pb", bufs=1) as pb, \
                     tc.tile_pool(name="pb2", bufs=2) as pb2:
                    atn = [pb.tile([P, TB * P], F16, tag=f"atn{h}",
                                   name=f"atn{h}") for h in range(2)]
                    with tc.tile_pool(name="psBs", bufs=2, space="PSUM") as psBs, \
                         tc.tile_pool(name="psBt", bufs=1, space="PSUM") as psBt, \
                         tc.tile_pool(name="psAv", bufs=1, space="PSUM") as psAv, \
                         tc.tile_pool(name="psBp", bufs=1, space="PSUM") as psBp:
                        for qc in range(TB):
                            W = (qc + 1) * P
                            probs_h = []
                            for h in range(2):
                                sc = psBs.tile([P, T], F32, tag="sc")
                                for c0 in range(0, W, 512):
                                    c1 = min(c0 + 512, W)
                                    nc.tensor.matmul(
                                        sc[:, c0:c1],
                                        qT[h][:, qc * P:(qc + 1) * P],
                                        kT[:, c0:c1], start=True, stop=True)
                                nc.vector.tensor_tensor(
                                    out=sc[:, W - P:W], in0=sc[:, W - P:W],
                                    in1=diagm[:], op=ALU.add)
                                probs = pb2.tile([P, T], F16, tag="probs",
                                                 bufs=4)
                                ssum = pb2.tile([P, 1], F32, tag="esum")
                                nc.scalar.activation(probs[:, :W], sc[:, :W],
                                                     AF.Exp,
                                                     accum_out=ssum[:, :1])
                                rec = pb2.tile([P, 1], F32, tag="rec")
                                nc.vector.reciprocal(rec[:], ssum[:])
                                nc.vector.tensor_scalar_mul(probs[:, :W],
                                                            probs[:, :W],
                                                            rec[:, :1])
                                probs_h.append(probs)
                            oTb = []
                            av = psAv.tile([P, 2 * P], F32, tag="av",
                                           name=f"av_{qc}")
                            for h in range(2):
                                probs = probs_h[h]
                                for g4 in range(0, qc + 1, 4):
                                    cnt = min(4, qc + 1 - g4)
                                    tp = psBt.tile([P, 512], F16, tag="tpB")
                                    for i in range(cnt):
                                        kc = g4 + i
                                        nc.tensor.transpose(
                                            tp[:, i * P:(i + 1) * P],
                                            probs[:, kc * P:(kc + 1) * P],
                                            identh[:])
                                    dst = atn[h][:, g4 * P:(g4 + cnt) * P]
                                    if (qc + h) % 2 == 0:
                                        nc.vector.tensor_copy(dst,
                                                              tp[:, :cnt * P])
                                    else:
                                        nc.scalar.activation(dst,
                                                             tp[:, :cnt * P],
                                                             AF.Copy)
                                for kc in range(qc + 1):
                                    nc.tensor.matmul(
                                        av[:, h * P:(h + 1) * P],
                                        v_tm[:, kc * P:(kc + 1) * P],
                                        atn[h][:, kc * P:(kc + 1) * P],
                                        start=(kc == 0), stop=(kc == qc))
                                ot = pb2.tile([P, P], F16, tag=f"oTb{h}")
                                if h == 0:
                                    nc.vector.tensor_copy(
                                        ot[:], av[:, h * P:(h + 1) * P])
                                else:
                                    nc.scalar.activation(
                                        ot[:], av[:, h * P:(h + 1) * P],
                                        AF.Copy)
                                oTb.append(ot)
                            q_ = qc // 4
                            ro = (qc % 4) * P
                            ob = pb2.tile([P, H], F16, tag="ob")
                            for nn in range(2):
                                ps = psBp.tile([P, 1024], F32, tag="psO")
                                for h in range(2):
                                    for q2 in range(2):
                                        s2 = slice(q2 * 512, (q2 + 1) * 512)
                                        nc.tensor.matmul(
                                            ps[:, s2], oTb[h][:],
                                            wo_sb[h][:, nn * 1024 + q2 * 512:
                                                      nn * 1024 + (q2 + 1) * 512],
                                            start=(h == 0), stop=(h == 1))
                                dst = ob[:, nn * 1024:(nn + 1) * 1024]
                                if nn == 0:
                                    nc.vector.tensor_copy(dst, ps[:])
                                else:
                                    nc.scalar.activation(dst, ps[:], AF.Copy)
                            eng = nc.sync if qc % 2 == 0 else nc.scalar
                            eng.dma_start(rs1_in[q_][ro:ro + P, :], ob[:])
                            if qc % 4 == 3:
                                nc.gpsimd.collective_compute(
                                    "ReduceScatter", ALU.add,
                                    ins=[rs1_in[q_].opt()],
                                    outs=[rs1_out[q_].opt()],
                                    replica_groups=RG)

            # ======== Phase D: residual + norm + local router ========
            with tc.tile_pool(name="pd", bufs=1) as pd:
                hid_sl = pd.tile([P, H], F32)
                nc.sync.dma_start(hid_sl[:], ex["hid_slice"][:])
                attn_sl = pd.tile([P, H], F16)
                for q in range(NCH):
                    nc.sync.dma_start(attn_sl[q * SH:(q + 1) * SH, :],
                                      rs1_out[q][:])
                res_sb = pd.tile([P, H], F32)
                nc.vector.tensor_add(res_sb[:], hid_sl[:], attn_sl[:])
                nc.sync.dma_start(res_slice[:], res_sb[:])
                dump2 = pd.tile([P, H], F32)
                ssum = pd.tile([P, 1], F32)
                nc.scalar.activation(dump2[:], res_sb[:], AF.Square,
                                     accum_out=ssum[:, :1])
                rms = pd.tile([P, 1], F32)
                nc.scalar.activation(rms[:], ssum[:], AF.Sqrt,
                                     bias=eps_t[:, :1], scale=1.0 / H)
                inv = pd.tile([P, 1], F32)
                nc.vector.reciprocal(inv[:], rms[:])
                x_sl = pd.tile([P, H], F32)
                nc.vector.tensor_scalar_mul(x_sl[:], res_sb[:], inv[:, :1])
                payx = pd.tile([P, H], F16)
                nc.vector.tensor_copy(payx[:], x_sl[:])
                nc.scalar.dma_start(agx1_in[:, 0:H // 2], payx[:, H // 2:H])
                nc.sync.dma_start(agx2_in[:], payx[:, 0:H // 2])
                # local router on fp32 x
                xsT = pd.tile([P, HC * P], F32)
                with tc.tile_pool(name="psDt", bufs=2, space="PSUM") as psDt:
                    for g4 in range(4):
                        tp = psDt.tile([P, 512], F32, tag="tpD")
                        for i in range(4):
                            hc = g4 * 4 + i
                            nc.tensor.transpose(
                                tp[:, i * P:(i + 1) * P],
                                x_sl[:, hc * P:(hc + 1) * P], identf[:])
                        nc.vector.tensor_copy(
                            xsT[:, g4 * 512:(g4 + 1) * 512], tp[:])
                with tc.tile_pool(name="psDr", bufs=1, space="PSUM") as psDr:
                    lg = psDr.tile([P, E], F32, tag="lg")
                    for hc in range(HC):
                        nc.tensor.matmul(lg[:], xsT[:, hc * P:(hc + 1) * P],
                                         gwr[:, hc * E:(hc + 1) * E],
                                         start=(hc == 0), stop=(hc == HC - 1))
                    sig = pd.tile([P, E], F32)
                    nc.scalar.activation(sig[:], lg[:], AF.Sigmoid)
                sb_ = pd.tile([P, E], F32)
                nc.vector.tensor_add(sb_[:], sig[:], gate_b_sb[:])
                mx8 = pd.tile([P, 8], F32)
                nc.vector.max(out=mx8[:], in_=sb_[:])
                s1 = pd.tile([P, E], F32)
                nc.vector.tensor_tensor(out=s1[:], in0=sb_[:],
                                        in1=mx8[:, 0:1].to_broadcast([P, E]),
                                        op=ALU.is_equal)
                s2 = pd.tile([P, E], F32)
                nc.vector.tensor_tensor(out=s2[:], in0=sb_[:],
                                        in1=mx8[:, 1:2].to_broadcast([P, E]),
                                        op=ALU.is_equal)
                nc.vector.tensor_add(s1[:], s1[:], s2[:])
                nc.vector.tensor_scalar_min(s1[:], s1[:], 1.0)
                wa = pd.tile([P, E], F32)
                nc.vector.tensor_mul(wa[:], s1[:], sig[:])
                nrm = pd.tile([P, 1], F32)
                nc.vector.reduce_sum(nrm[:], wa[:], axis=AX.X)
                rec = pd.tile([P, 1], F32)
                nc.vector.reciprocal(rec[:], nrm[:])
                paw = pd.tile([P, WP], F16)
                nc.vector.tensor_scalar_mul(paw[:, 0:E], wa[:], rec[:, :1])
                selh = pd.tile([P, E], F16)
                nc.vector.tensor_copy(selh[:], s1[:])
                uml = pd.tile([P, E], F32)
                nc.vector.tensor_scalar(out=uml[:], in0=selh[:],
                                        scalar1=-BIG2, scalar2=BIG2,
                                        op0=ALU.mult, op1=ALU.add)
                pre_l = pd.tile([P, E], F16)
                with tc.tile_pool(name="psDp", bufs=1, space="PSUM") as psDp:
                    prp = psDp.tile([P, E], F32, tag="prp")
                    nc.tensor.matmul(prp[:], ut_sb[:], selh[:],
                                     start=True, stop=True)
                    nc.vector.tensor_copy(pre_l[:], prp[:])
                    nc.vector.tensor_tensor(out=paw[:, E:2 * E], in0=prp[:],
                                            in1=uml[:], op=ALU.add)
                    tbp = psDp.tile([P, E], F32, tag="tbp")
                    nc.tensor.matmul(tbp[:], bc127[:], pre_l[:],
                                     start=True, stop=True)
                    nc.vector.tensor_copy(paw[:, 2 * E:3 * E], tbp[:])
                nc.scalar.dma_start(agx1_in[:, H // 2:H // 2 + WP], paw[:])

            nc.gpsimd.collective_compute(
                "AllGather", ALU.bypass, ins=[agx1_in.opt()],
                outs=[x_tmA.opt()], replica_groups=RG)
            nc.gpsimd.collective_compute(
                "AllGather", ALU.bypass, ins=[agx2_in.opt()],
                outs=[x_tmB.opt()], replica_groups=RG)

            # ======== Phase X: token lists + x^T + shared + experts ======
            with tc.tile_pool(name="pg", bufs=1) as pg, \
                 tc.tile_pool(name="pg2", bufs=2) as pg2:
                xT = [pg.tile([P, T], F16, tag=f"xT{hc}", name=f"xT{hc}")
                      for hc in range(HC)]
                totals = pg.tile([8, E], F16)
                grank = [pg.tile([P, E], F32, tag=f"grank{b}",
                                 name=f"grank{b}") for b in range(TB)]
                wb0 = H // 2
                with tc.tile_pool(name="psXr", bufs=2, space="PSUM") as psXr:
                    nc.sync.dma_start(
                        totals[:].rearrange("b (o e) -> b o e", o=1),
                        x_tmA[:].rearrange("(b p) e -> b p e", p=P)[
                            :, 0:1, wb0 + 2 * E:wb0 + 3 * E])
                    wrbs = []
                    for b in range(TB):
                        wrb = pg2.tile([P, WP], F16, tag="wrb", bufs=8,
                                       name=f"wrb{b}")
                        nc.sync.dma_start(
                            wrb[:], x_tmA[b * P:(b + 1) * P, wb0:wb0 + WP])
                        wrbs.append(wrb)
                    for b in range(TB):
                        ofs = psXr.tile([P, E], F32, tag="ofs")
                        nc.tensor.matmul(ofs[:], slb_sb[:, b * P:(b + 1) * P],
                                         totals[:], start=True, stop=True)
                        nc.vector.tensor_tensor(out=grank[b][:],
                                                in0=wrbs[b][:, E:2 * E],
                                                in1=ofs[:], op=ALU.add)
                        gm = pg2.tile([P, E], F32, tag="gm")
                        nc.vector.tensor_scalar(out=gm[:], in0=grank[b][:],
                                                scalar1=float(CAP),
                                                scalar2=BIG,
                                                op0=ALU.is_gt, op1=ALU.mult)
                        nc.vector.tensor_add(grank[b][:], grank[b][:], gm[:])
                    sent = pg.tile([P, 1], I32)
                    nc.vector.memset(sent[:], 1000000)
                    for kk in range(2 * SL // P):
                        nc.sync.dma_start(tok_lists[kk * P:(kk + 1) * P, :],
                                          sent[:])
                    for b in range(TB):
                        tok = pg2.tile([P, 1], I32, tag="tok")
                        nc.gpsimd.iota(tok[:], pattern=[[0, 1]], base=b * P,
                                       channel_multiplier=1)
                        for ei in range(2):
                            gsel = pg2.tile([P, E], F32, tag="gsel")
                            nc.vector.tensor_mul(gsel[:], grank[b][:],
                                                 em[ei][:])
                            ridx = pg2.tile([P, 1], F32, tag="ridx")
                            nc.vector.reduce_sum(ridx[:], gsel[:], axis=AX.X)
                            nc.vector.tensor_scalar_add(ridx[:], ridx[:],
                                                        float(ei * SL - 1))
                            ridx_i = pg2.tile([P, 1], I32, tag="ridxi")
                            nc.vector.tensor_copy(ridx_i[:], ridx[:])
                            nc.gpsimd.indirect_dma_start(
                                out=tok_lists[:],
                                out_offset=bass.IndirectOffsetOnAxis(
                                    ap=ridx_i[:, :1], axis=0),
                                in_=tok[:], in_offset=None,
                                bounds_check=2 * SL - 1, oob_is_err=False)

                # x^T build: half A (hc 0-7) then half B (hc 8-15),
                # shared-expert gate pass interleaved
                hsh = [pg.tile([P, T], F16, tag=f"hs{sp}", name=f"hs{sp}")
                       for sp in range(2)]
                gsh = [pg.tile([P, T], F16, tag=f"gsh{sp}", name=f"gsh{sp}")
                       for sp in range(2)]
                with tc.tile_pool(name="psXt", bufs=2, space="PSUM") as psXt, \
                     tc.tile_pool(name="psS", bufs=1, space="PSUM") as psS:
                    gps = [psS.tile([P, T], F32, tag=f"sgp{sp}",
                                    name=f"sgp{sp}") for sp in range(2)]
                    for half, src_tm, hclo in [(0, x_tmB, 0), (1, x_tmA, 8)]:
                        for n in range(2):
                            xbs = []
                            for bb in range(4):
                                b = n * 4 + bb
                                xb = pg2.tile([P, H // 2], F16, tag="xb",
                                              bufs=4, name=f"xb{half}_{b}")
                                eng = nc.sync if bb % 2 == 0 else nc.scalar
                                eng.dma_start(
                                    xb[:], src_tm[b * P:(b + 1) * P,
                                                  0:H // 2])
                                xbs.append(xb)
                            sl = slice(n * 512, (n + 1) * 512)
                            for hc8 in range(8):
                                hc = hclo + hc8
                                tp = psXt.tile([P, 512], F16, tag="tpX")
                                for bb in range(4):
                                    nc.tensor.transpose(
                                        tp[:, bb * P:(bb + 1) * P],
                                        xbs[bb][:, hc8 * P:(hc8 + 1) * P],
                                        identh[:])
                                if hc % 2 == 0:
                                    nc.vector.tensor_copy(xT[hc][:, sl], tp[:])
                                else:
                                    nc.scalar.activation(xT[hc][:, sl], tp[:],
                                                         AF.Copy)
                        # shared gate pass for this half's hc range
                        for hc8 in range(8):
                            hc = hclo + hc8
                            for sp in range(2):
                                c0 = hc * ISC + sp * P
                                for nn in range(2):
                                    sl = slice(nn * 512, (nn + 1) * 512)
                                    nc.tensor.matmul(gps[sp][:, sl],
                                                     wsg_sb[:, c0:c0 + P],
                                                     xT[hc][:, sl],
                                                     start=(hc == 0),
                                                     stop=(hc == HC - 1))
                    for sp in range(2):
                        nc.vector.tensor_copy(gsh[sp][:], gps[sp][:])
                        nc.scalar.activation(gsh[sp][:], gsh[sp][:], AF.Silu)
                    ups = [psS.tile([P, T], F32, tag=f"sgp{sp}",
                                    name=f"sup{sp}") for sp in range(2)]
                    for hc in range(HC):
                        for sp in range(2):
                            c0 = hc * ISC + sp * P
                            for nn in range(2):
                                sl = slice(nn * 512, (nn + 1) * 512)
                                nc.tensor.matmul(ups[sp][:, sl],
                                                 wsu_sb[:, c0:c0 + P],
                                                 xT[hc][:, sl],
                                                 start=(hc == 0),
                                                 stop=(hc == HC - 1))
                    for sp in range(2):
                        nc.vector.tensor_mul(hsh[sp][:], gsh[sp][:],
                                             ups[sp][:])
                with tc.tile_pool(name="psS3", bufs=2, space="PSUM") as psS3:
                    for nn in range(2):
                        for tb2 in range(TB // 2):
                            sd = pg2.tile([P, 2048], F16, tag="sd", bufs=1)
                            for two in range(2):
                                tb_ = tb2 * 2 + two
                                ps3 = psS3.tile([P, 1024], F32, tag="psSd")
                                for sp in range(2):
                                    for q2 in range(2):
                                        s2 = slice(q2 * 512, (q2 + 1) * 512)
                                        nc.tensor.matmul(
                                            ps3[:, s2],
                                            hsh[sp][:, tb_ * P:(tb_ + 1) * P],
                                            wsd_sb[sp][:, nn * 1024 + q2 * 512:
                                                        nn * 1024 + (q2 + 1) * 512],
                                            start=(sp == 0), stop=(sp == 1))
                                dst = sd[:, two * 1024:(two + 1) * 1024]
                                if two == 0:
                                    nc.vector.tensor_copy(dst, ps3[:])
                                else:
                                    nc.scalar.activation(dst, ps3[:], AF.Copy)
                            eng = nc.sync if tb2 % 2 == 0 else nc.scalar
                            eng.dma_start(
                                rs2_in[nn][tb2 * 2 * P:(tb2 + 1) * 2 * P, :]
                                .rearrange("(two p) c -> p two c", two=2),
                                sd[:].rearrange("p (two c) -> p two c", two=2))

                # ======== Expert gathers + wcol (gpsimd ahead of PE) ======
                KL = [P, CAP - P]
                idx_sb = [[pg.tile([P if k == 0 else CAP - P, 1], I32,
                                   tag=f"idx{ei}_{k}",
                                   name=f"idx{ei}_{k}") for k in range(2)]
                          for ei in range(2)]
                gxT = [pg.tile([P, HC * SL], F16, tag=f"gxT{ei}",
                               name=f"gxT{ei}") for ei in range(2)]
                wcol = [[pg.tile([P if k == 0 else CAP - P, 1], F32,
                                 tag=f"wcol{ei}_{k}",
                                 name=f"wcol{ei}_{k}") for k in range(2)]
                        for ei in range(2)]
                with tc.tile_pool(name="psFt", bufs=2, space="PSUM") as psFt:
                    for ei in range(2):
                        gxA = [None, None]
                        gxB = [None, None]
                        for k in range(2):
                            nc.sync.dma_start(
                                idx_sb[ei][k][:],
                                tok_lists[ei * SL + k * P:
                                          ei * SL + k * P + KL[k], :])
                            ga_ = pg2.tile([P, H // 2 + WP], F16, tag="gxA",
                                           name=f"gxA{ei}_{k}")
                            nc.vector.memset(ga_[:KL[k], :], 0.0)
                            nc.gpsimd.indirect_dma_start(
                                out=ga_[:KL[k], :], out_offset=None,
                                in_=x_tmA[:],
                                in_offset=bass.IndirectOffsetOnAxis(
                                    ap=idx_sb[ei][k][:, :1], axis=0),
                                bounds_check=T - 1, oob_is_err=False)
                            gb_ = pg2.tile([P, H // 2], F16, tag="gxB",
                                           name=f"gxB{ei}_{k}")
                            nc.vector.memset(gb_[:KL[k], :], 0.0)
                            nc.gpsimd.indirect_dma_start(
                                out=gb_[:KL[k], :], out_offset=None,
                                in_=x_tmB[:],
                                in_offset=bass.IndirectOffsetOnAxis(
                                    ap=idx_sb[ei][k][:, :1], axis=0),
                                bounds_check=T - 1, oob_is_err=False)
                            gxA[k] = ga_
                            gxB[k] = gb_
                        # wcol first (vector-only, unblocks nothing behind)
                        for k in range(2):
                            wtmp_f = pg2.tile([P, E], F32, tag="wtmp")
                            wtmp = wtmp_f[:KL[k], :]
                            nc.vector.tensor_mul(
                                wtmp, gxA[k][:KL[k], wb0:wb0 + E],
                                em[ei][:KL[k], :])
                            nc.vector.reduce_sum(wcol[ei][k][:], wtmp,
                                                 axis=AX.X)
                        for hp in range(HC // 2):
                            tp = psFt.tile([P, 2 * CAP], F16, tag="tpF")
                            for i in range(2):
                                hc = hp * 2 + i
                                o0 = i * CAP
                                gsrc = gxB if hc < 8 else gxA
                                c0 = (hc % 8) * P
                                nc.tensor.transpose(
                                    tp[:, o0:o0 + P],
                                    gsrc[0][:, c0:c0 + P],
                                    identh[:])
                                nc.tensor.transpose(
                                    tp[:, o0 + P:o0 + CAP],
                                    gsrc[1][:KL[1], c0:c0 + P],
                                    identh[:KL[1], :KL[1]])
                            dst = gxT[ei][:].rearrange(
                                "p (hc c) -> p hc c", hc=HC)[
                                :, hp * 2:hp * 2 + 2, 0:CAP]
                            src = tp[:].rearrange("p (hc c) -> p hc c", hc=2)
                            if hp % 2 == 0:
                                nc.vector.tensor_copy(dst, src)
                            else:
                                nc.scalar.activation(dst, src, AF.Copy)

                # ======== Experts: I-partitioned gate/up ======
                h_sb = [pg.tile([P, IP * CAP], F16, tag=f"h_sb{ei}",
                                name=f"h_sb{ei}") for ei in range(2)]
                sg_sb = pg.tile([P, IP * CAP], F16)
                wd_res = [pg.tile([P, H], F16, tag=f"wd{e}_{ip}",
                                  name=f"wd{e}_{ip}")
                          for e in range(2) for ip in range(IP)]
                for ip in range(IP):
                    nc.gpsimd.dma_start(wd_res[ip][:],
                                        ex["we_d"][0, ip * P:(ip + 1) * P, :])
                wi = 2
                for ei in range(2):
                    if ei == 1:
                        for ip in range(IP):
                            nc.gpsimd.dma_start(
                                wd_res[IP + ip][:],
                                ex["we_d"][1, ip * P:(ip + 1) * P, :])
                    with tc.tile_pool(name=f"psF1{ei}", bufs=1,
                                      space="PSUM") as psF1:
                        acc = [psF1.tile([P, 256], F32, tag=f"acc{ip}",
                                         name=f"acc{ip}_{ei}")
                               for ip in range(IP)]
                        for kind in ("g", "u"):
                            base = (0 if kind == "g" else NJ) + ei * 2 * NJ
                            for j in range(NJ):
                                wp = wring[base + j]
                                if wi < len(wseq):
                                    issue_pair(wi)
                                    wi += 1
                                for four in range(4):
                                    hc = 4 * j + four
                                    for ip in range(IP):
                                        nc.tensor.matmul(
                                            acc[ip][:, :CAP],
                                            wp[:, four * I + ip * P:
                                               four * I + (ip + 1) * P],
                                            gxT[ei][:, hc * SL:hc * SL + CAP],
                                            start=(hc == 0),
                                            stop=(hc == HC - 1))
                            if kind == "g":
                                for ip in range(IP):
                                    nc.scalar.activation(
                                        sg_sb[:, ip * CAP:(ip + 1) * CAP],
                                        acc[ip][:, :CAP], AF.Silu)
                            else:
                                for ip in range(IP):
                                    nc.vector.tensor_mul(
                                        h_sb[ei][:, ip * CAP:(ip + 1) * CAP],
                                        sg_sb[:, ip * CAP:(ip + 1) * CAP],
                                        acc[ip][:, :CAP])

                # ======== Down-projections, column-half outer + RS2 ======
                with tc.tile_pool(name="psF3", bufs=2, space="PSUM") as psF3:
                    for nn in range(2):
                        for ei in range(2):
                            for k in range(2):
                                kl = KL[k]
                                koff = k * P
                                psd = psF3.tile([P, 1024], F32, tag="fd")
                                for ip in range(IP):
                                    c0 = ip * CAP + koff
                                    for q2 in range(2):
                                        s2 = slice(q2 * 512, (q2 + 1) * 512)
                                        nc.tensor.matmul(
                                            psd[:kl, s2],
                                            h_sb[ei][:, c0:c0 + kl],
                                            wd_res[ei * IP + ip][
                                                :, nn * 1024 + q2 * 512:
                                                nn * 1024 + (q2 + 1) * 512],
                                            start=(ip == 0),
                                            stop=(ip == IP - 1))
                                out_f = pg2.tile([P, 1024], F16, tag="outsb",
                                                 name=f"outsb{nn}{ei}{k}")
                                nc.vector.tensor_scalar_mul(
                                    out_f[:kl, :], psd[:kl, :],
                                    wcol[ei][k][:, :1])
                                nc.gpsimd.indirect_dma_start(
                                    out=rs2_in[nn][:],
                                    out_offset=bass.IndirectOffsetOnAxis(
                                        ap=idx_sb[ei][k][:, :1], axis=0),
                                    in_=out_f[:kl, :], in_offset=None,
                                    bounds_check=T - 1, oob_is_err=False,
                                    compute_op=ALU.add)
                        nc.gpsimd.collective_compute(
                            "ReduceScatter", ALU.add, ins=[rs2_in[nn].opt()],
                            outs=[rs2_out[nn].opt()], replica_groups=RG)

            for nn in range(2):
                nc.sync.dma_start(out_slice[:, nn * 1024:(nn + 1) * 1024],
                                  rs2_out[nn][:])


_CACHE = {}


def _build():
    key = "nc"
    if key in _CACHE:
        return _CACHE[key]
    nc = bacc.Bacc("TRN2", target_bir_lowering=False, debug=False,
                   num_devices=NCN)
    with tile.TileContext(nc) as tc:
        _emit(nc, tc)
    nc.compile()
    _CACHE[key] = nc
    return nc


def _perm_rows(c):
    return np.concatenate([np.arange(q * CH + c * SH, q * CH + (c + 1) * SH)
                           for q in range(NCH)])


def _host_prep(inputs):
    f16 = np.float16
    pos = np.asarray(inputs["positions"]).astype(np.float64)
    hid = np.asarray(inputs["hidden_states"], np.float32)
    w_in = np.asarray(inputs["w_in_ln"], np.float32)
    w_post = np.asarray(inputs["w_post_ln"], np.float32)
    wq = np.asarray(inputs["wq"], np.float32) * w_in[:, None]
    wk = np.asarray(inputs["wk"], np.float32) * w_in[:, None]
    wv = np.asarray(inputs["wv"], np.float32) * w_in[:, None]
    wo = np.asarray(inputs["wo"], np.float32)
    gate_w = np.asarray(inputs["gate_w"], np.float32) * w_post[None, :]
    gate_b = np.asarray(inputs["gate_bias"], np.float32).reshape(1, E)
    we_g = (np.asarray(inputs["we_gate"], np.float32)
            * w_post[None, :, None]).astype(f16)
    we_u = (np.asarray(inputs["we_up"], np.float32)
            * w_post[None, :, None]).astype(f16)
    we_d = np.asarray(inputs["we_down"], np.float32).astype(f16)
    ws_g = np.asarray(inputs["ws_gate"], np.float32) * w_post[:, None]
    ws_u = np.asarray(inputs["ws_up"], np.float32) * w_post[:, None]
    ws_d = np.asarray(inputs["ws_down"], np.float32).astype(f16)

    inv_freq = 1.0 / (THETA ** (np.arange(0, D, 2, dtype=np.float64) / D))
    f = pos[None, :] * inv_freq[:, None]
    cos2, sin2 = np.cos(f), np.sin(f)
    cosT = np.repeat(cos2, 2, axis=0).astype(np.float32)
    sinT = np.empty((D, T), np.float32)
    sinT[0::2] = -sin2
    sinT[1::2] = sin2
    s = 1.0 / np.sqrt(D)
    cosq, sinq = (cosT * s).astype(f16), (sinT * s).astype(f16)
    cosk, sink = cosT.astype(f16), sinT.astype(f16)

    ii = np.arange(P)
    diagmask = np.where(ii[:, None] >= ii[None, :], 0.0, NEG).astype(f16)
    ident = np.eye(P, dtype=np.float32)
    ut_in = np.triu(np.ones((P, P), np.float32)).astype(f16)
    slb_in = np.zeros((8, TB * P), np.float32)
    for b in range(TB):
        slb_in[:b, b * P:(b + 1) * P] = 1.0
    slb_in = slb_in.astype(f16)
    bc127 = np.zeros((P, P), np.float32)
    bc127[127, :] = 1.0
    bc127 = bc127.astype(f16)
    perm = np.zeros((P, P), np.float32)
    for i in range(0, P, 2):
        perm[i, i + 1] = 1.0
        perm[i + 1, i] = 1.0

    def pack_pk(w, width):  # w: [H, width]
        return np.ascontiguousarray(
            w.reshape(HC, P, width).transpose(1, 0, 2).reshape(P, HC * width))

    gate_w_pk = pack_pk(gate_w.T.astype(np.float32), E)

    maps = []
    for c in range(NCN):
        g = c // 2
        w_qkv = pack_pk(np.concatenate([
            wq[:, 2 * c * D:(2 * c + 1) * D],
            wq[:, (2 * c + 1) * D:(2 * c + 2) * D],
            wk[:, g * D:(g + 1) * D],
            wv[:, g * D:(g + 1) * D]], axis=1), 512).astype(f16)
        em0 = np.zeros((P, E), np.float32)
        em0[:, 2 * c] = 1.0
        em1 = np.zeros((P, E), np.float32)
        em1[:, 2 * c + 1] = 1.0
        maps.append({
            "hid": hid.astype(f16),
            "hid_slice": np.ascontiguousarray(hid[_perm_rows(c)]),
            "w_qkv_pk": w_qkv,
            "wo0": np.ascontiguousarray(wo[2 * c * D:(2 * c + 1) * D]).astype(f16),
            "wo1": np.ascontiguousarray(
                wo[(2 * c + 1) * D:(2 * c + 2) * D]).astype(f16),
            "cosq": cosq, "sinq": sinq, "cosk": cosk, "sink": sink,
            "permh": perm.astype(f16), "identh_in": ident.astype(f16),
            "identr_in": ident, "diagmask": diagmask,
            "gate_w_pk": gate_w_pk,
            "gate_b": np.broadcast_to(gate_b, (P, E)).astype(np.float32).copy(),
            "emask0": em0, "emask1": em1,
            "ut_in": ut_in, "slb_in": slb_in, "bcast127": bc127,
            "ws_g_pk": pack_pk(
                ws_g[:, c * ISC:(c + 1) * ISC].astype(np.float32), ISC
            ).astype(f16),
            "ws_u_pk": pack_pk(
                ws_u[:, c * ISC:(c + 1) * ISC].astype(np.float32), ISC
            ).astype(f16),
            "ws_d": np.ascontiguousarray(ws_d[c * ISC:(c + 1) * ISC]),
            "we_g": np.ascontiguousarray(we_g[2 * c:2 * c + 2]),
            "we_u": np.ascontiguousarray(we_u[2 * c:2 * c + 2]),
            "we_d": np.ascontiguousarray(we_d[2 * c:2 * c + 2]),
        })
    return maps


def kernel(trace=False, **inputs):
    nc = _build()
    maps = _host_prep(inputs)
    res = bass_utils.run_bass_kernel_spmd(
        nc, maps, core_ids=list(range(NCN)), trace=trace)
    out = np.empty((T, H), np.float32)
    resid = np.empty((T, H), np.float32)
    for c in range(NCN):
        rows = _perm_rows(c)
        out[rows] = res.results[c]["out_slice"].astype(np.float32)
        resid[rows] = res.results[c]["res_slice"]
    kernel.last_results = res
    return out, resid
